# revision 1
# baseline (speedup 1.0000x reference)
"""Trainium2 Bass kernel for a 3x3 VALID conv: x[64,256,256] * k[128,64,3,3] -> [128,254,254].

Strategy:
  - Shard output rows across 8 cores (32 rows each; 8*32=256 >= 254, tail padded).
  - Per core, conv is 6 accumulated matmuls per pair of output rows:
      contraction K=128 = 64 in-channels x 2 kernel rows (kh=0,1 packed in the
      partition dim via a row-shifted duplicate of x on partitions 64..127);
      kh=2 runs as 3 more K=128 matmuls whose lower-half weights are zero.
    M=128 output channels, N=508 = 2 output rows x 254 cols (one PSUM bank).
  - PSUM evacuation fused with the bias add on the Vector engine.
  - Host gathers the 8 per-core output slabs.
"""

import os
import sys

import numpy as np

for _p in ("/opt/trn_rl_repo", "/root/.axon_site/_ro/trn_rl_repo"):
    if os.path.isdir(_p) and _p not in sys.path:
        sys.path.insert(0, _p)

from concourse import bass, mybir, tile  # noqa: E402
from concourse.bass_utils import run_bass_kernel_spmd  # noqa: E402

IN_C, H, W = 64, 256, 256
KS = 3
OUT_C = 128
OH, OW = H - KS + 1, W - KS + 1  # 254, 254
N_CORES = 8
RPC = 32          # output rows computed per core (8*32 = 256 >= 254)
PAD_H = 259       # padded input rows so core 7 can read h0+34 = 258

# x lives in one SBUF tile of Q q-rows, loaded by several region DMAs (Tile's
# dep tracking is region-precise, so pair p's matmuls only wait on the slices
# covering q in [2p, 2p+3]; the wait-splitter legalizes multi-slice waits).
Q = RPC + 2       # 34 q-rows, zero halo
LOAD_ROWS = 4     # q-rows per load slice

# Matmul dtype: "f32r" (full-rate fp32-ish), "bf16", or "f32" (exact, 4x slower)
MM_DT = os.environ.get("CONV_MM_DT", "f32r")

TRACE = False
LAST_RESULTS = None

_COMPILED = {}


def _np_dt(mm_dt):
    if mm_dt == "bf16":
        import ml_dtypes

        return np.dtype(ml_dtypes.bfloat16)
    return np.dtype(np.float32)


def _bass_dt(mm_dt):
    return {
        "bf16": mybir.dt.bfloat16,
        "f32r": mybir.dt.float32r,
        "f32": mybir.dt.float32,
    }[mm_dt]


def _build_program(mm_dt):
    dt = _bass_dt(mm_dt)
    f32 = mybir.dt.float32
    nc = bass.Bass()

    x_ext = nc.declare_dram_parameter("xdup", [128, Q * W], dt, isOutput=False)
    w_ext = nc.declare_dram_parameter("wpack", [128, 6 * 128], dt, isOutput=False)
    b_ext = nc.declare_dram_parameter("bias", [128, 1], f32, isOutput=False)
    o_ext = nc.declare_dram_parameter("out", [128, RPC * OW], f32, isOutput=True)

    with tile.TileContext(nc) as tc:
        n_pairs = RPC // 2
        with (
            tc.tile_pool(name="wpool", bufs=1) as wpool,
            tc.tile_pool(name="xpool", bufs=1) as xpool,
            tc.tile_pool(name="pspool", bufs=4, space="PSUM") as pspool,
            # bufs = n pairs: output tiles are never reused -> evacuations
            # only ever wait on their PSUM producer.
            tc.tile_pool(name="opool", bufs=n_pairs) as opool,
        ):
            # Loads dispatch from the ACT HWDGE sequencer, stores from SP:
            # a store's hoisted DVE wait then never stalls a load dispatch.
            wt = wpool.tile([128, 6 * 128], dt)
            nc.scalar.dma_start(out=wt[:], in_=w_ext[:])
            bt = wpool.tile([128, 1], f32)
            nc.scalar.dma_start(out=bt[:], in_=b_ext[:])

            wv = wt[:].rearrange("p (s m) -> p s m", m=128)
            ov = o_ext.rearrange("p (r w) -> p r w", w=OW)

            xt = xpool.tile([128, Q * W], dt)
            for q0 in range(0, Q, LOAD_ROWS):
                q1 = min(q0 + LOAD_ROWS, Q)
                nc.scalar.dma_start(
                    out=xt[:, q0 * W : q1 * W], in_=x_ext[:, q0 * W : q1 * W]
                )
            xv = xt[:].rearrange("p (q w) -> p q w", w=W)

            for lp in range(n_pairs):
                r = 2 * lp
                ps = pspool.tile([128, 2 * OW], f32)
                for j in range(6):
                    kw = j % 3
                    q0 = r if j < 3 else r + 2
                    nc.tensor.matmul(
                        ps[:],
                        lhsT=wv[:, j, :],
                        rhs=xv[:, q0 : q0 + 2, kw : kw + OW],
                        start=(j == 0),
                        stop=(j == 5),
                    )
                so = opool.tile([128, 2 * OW], f32)
                nc.vector.tensor_scalar_add(so[:], ps[:], bt[:, 0:1])
                nc.sync.dma_start(out=ov[:, r : r + 2, :], in_=so[:])

    _split_multi_waits(nc)
    return nc


def _split_multi_waits(nc):
    """Walrus codegen accepts a single sync-wait command per instruction.

    Tile's sem assignment happily attaches several. Hoist all but the last
    wait of every instruction onto fresh NoOps placed immediately before it
    on the same engine stream (engine streams execute in program order, so
    semantics are preserved; the wait merely moves from the instruction to
    its dispatching sequencer).
    """
    for fn in nc.m.functions:
        for bb in fn.blocks:
            out = []
            for inst in bb.instructions:
                si = inst.sync_info
                waits = list(si.on_wait) if si is not None and si.on_wait else []
                if len(waits) > 1:
                    for wt_ in waits[:-1]:
                        nop = mybir.InstNoOp(
                            name=nc.get_next_instruction_name(),
                            engine=inst.engine,
                        )
                        nop.sync_info = mybir.SyncInfo(
                            on_wait=[wt_], on_update=[]
                        )
                        nc.register_instruction(nop)
                        out.append(nop)
                    inst.sync_info = mybir.SyncInfo(
                        on_wait=[waits[-1]], on_update=list(si.on_update)
                    )
                out.append(inst)
            bb.instructions = out


def _get_program(mm_dt):
    if mm_dt not in _COMPILED:
        _COMPILED[mm_dt] = _build_program(mm_dt)
    return _COMPILED[mm_dt]


def _prep_inputs(x, kernels, biases, mm_dt):
    np_dt = _np_dt(mm_dt)
    xp = np.zeros((IN_C, PAD_H, W), dtype=np.float32)
    xp[:, :H] = x
    xp = xp.astype(np_dt)

    # wpack[:, s, :]: s=kw -> (kh0 on partitions 0..63, kh1 on 64..127);
    # s=3+kw -> (kh2 on 0..63, zeros on 64..127).
    wpack = np.zeros((128, 6, 128), dtype=np.float32)
    for kw in range(KS):
        wpack[:64, kw, :] = kernels[:, :, 0, kw].T
        wpack[64:, kw, :] = kernels[:, :, 1, kw].T
        wpack[:64, 3 + kw, :] = kernels[:, :, 2, kw].T
    wpack = wpack.reshape(128, 6 * 128).astype(np_dt)

    bias = np.ascontiguousarray(biases.astype(np.float32).reshape(128, 1))

    in_maps = []
    for core in range(N_CORES):
        h0 = RPC * core
        xdup = np.empty((128, Q, W), dtype=np_dt)
        xdup[:64] = xp[:, h0 : h0 + Q]
        xdup[64:] = xp[:, h0 + 1 : h0 + 1 + Q]
        in_maps.append(
            {
                "xdup": xdup.reshape(128, Q * W),
                "wpack": wpack,
                "bias": bias,
            }
        )
    return in_maps


def kernel(x, kernels, biases):
    global LAST_RESULTS
    x = np.asarray(x, dtype=np.float32)
    kernels = np.asarray(kernels, dtype=np.float32)
    biases = np.asarray(biases, dtype=np.float32)

    nc = _get_program(MM_DT)
    in_maps = _prep_inputs(x, kernels, biases, MM_DT)
    res = run_bass_kernel_spmd(nc, in_maps, core_ids=list(range(N_CORES)), trace=TRACE)
    LAST_RESULTS = res

    out = np.empty((OUT_C, N_CORES * RPC, OW), dtype=np.float32)
    for c in range(N_CORES):
        out[:, RPC * c : RPC * (c + 1), :] = res.results[c]["out"].reshape(
            OUT_C, RPC, OW
        )
    return np.ascontiguousarray(out[:, :OH, :])



# revision 3
# speedup vs baseline: 1.3468x; 1.3468x over previous
"""Trainium2 Bass kernel for a 3x3 VALID conv: x[64,256,256] * k[128,64,3,3] -> [128,254,254].

Strategy (v2):
  - Shard output rows across 8 cores (32 rows each; 8*32=256 >= 254, tail padded).
  - bf16 matmuls + bf16 x/w/out DMA (fp32 PSUM accumulate): rel err ~4e-3,
    half the HBM traffic of fp32.
  - 5 accumulated matmuls per pair of output rows (vs 6 for the naive
    two-kernel-row packing), using two SBUF layouts of x:
      xdup  [128, 32*256]: partition half0 = x[q],   half1 = x[q+1]
        -> passes kw=0,1,2 each cover taps (0,kw)+(1,kw)       (3 passes)
      xdup2 [128, 32*256]: partition half0 = x[q+2], half1 = x[q+2] shifted
        one column left (pad 0)
        -> one pass covers taps (2,0)+(2,1); one half-pass covers (2,2)
    M=128 output channels, N=508 = 2 output rows x 254 cols (one PSUM bank).
  - PSUM evacuation fused with the bias add on the Vector engine, writing
    bf16; host converts to f32.
  - Dummy warm-up matmuls at t~0 keep the PE p-state ramp running while the
    first x slices load, so real matmuls run at full clock sooner.
  - Host gathers the 8 per-core output slabs.
"""

import os
import sys

import numpy as np

for _p in ("/opt/trn_rl_repo", "/root/.axon_site/_ro/trn_rl_repo"):
    if os.path.isdir(_p) and _p not in sys.path:
        sys.path.insert(0, _p)

from concourse import bass, mybir, tile  # noqa: E402
from concourse.bass_utils import run_bass_kernel_spmd  # noqa: E402

IN_C, H, W = 64, 256, 256
KS = 3
OUT_C = 128
OH, OW = H - KS + 1, W - KS + 1  # 254, 254
N_CORES = 8
RPC = 32          # output rows computed per core (8*32 = 256 >= 254)
PAD_H = 259       # padded input rows so core 7 can read h0+33+2 = 258
Q = RPC           # q-rows per layout tile
LOAD_ROWS = 4     # q-rows per load slice

N_WARM = int(os.environ.get("CONV_N_WARM", "14"))
WARM_AP = 128
MM_DT = "bf16"  # informational; v2 is bf16-only

TRACE = False
LAST_RESULTS = None

_COMPILED = {}

_BF16 = None


def _np_bf16():
    global _BF16
    if _BF16 is None:
        import ml_dtypes

        _BF16 = np.dtype(ml_dtypes.bfloat16)
    return _BF16


def _build_program():
    dt = mybir.dt.bfloat16
    f32 = mybir.dt.float32
    nc = bass.Bass()

    x1_ext = nc.declare_dram_parameter("xdup", [128, Q * W], dt, isOutput=False)
    x2_ext = nc.declare_dram_parameter("xdup2", [128, Q * W], dt, isOutput=False)
    w_ext = nc.declare_dram_parameter("wpack", [128, 5 * 128], dt, isOutput=False)
    b_ext = nc.declare_dram_parameter("bias", [128, 1], f32, isOutput=False)
    o_ext = nc.declare_dram_parameter("out", [128, RPC * OW], dt, isOutput=True)

    with tile.TileContext(nc) as tc:
        n_pairs = RPC // 2
        with (
            tc.tile_pool(name="wpool", bufs=1) as wpool,
            tc.tile_pool(name="xpool", bufs=1) as xpool,
            tc.tile_pool(name="pspool", bufs=6, space="PSUM") as pspool,
            tc.tile_pool(name="wmpool", bufs=1, space="PSUM") as wmpool,
            # bufs = n pairs: output tiles are never reused -> evacuations
            # only ever wait on their PSUM producer.
            tc.tile_pool(name="opool", bufs=n_pairs) as opool,
        ):
            # PE p-state warm-up: tiny memset then dummy matmuls that keep the
            # Tensor engine continuously busy while the first loads land.
            if N_WARM:
                wmt = wpool.tile([128, WARM_AP], dt)
                nc.vector.memset(wmt[:], 0.0)
                psw = wmpool.tile([128, WARM_AP], f32)
                for _ in range(N_WARM):
                    nc.tensor.matmul(
                        psw[:], lhsT=wmt[:, 0:128], rhs=wmt[:], start=True, stop=True
                    )

            # Loads dispatch from the ACT HWDGE sequencer, stores from SP:
            # a store's hoisted DVE wait then never stalls a load dispatch.
            wt = wpool.tile([128, 5 * 128], dt)
            nc.scalar.dma_start(out=wt[:], in_=w_ext[:])
            bt = wpool.tile([128, 1], f32)
            nc.scalar.dma_start(out=bt[:], in_=b_ext[:])

            wv = wt[:].rearrange("p (s m) -> p s m", m=128)
            ov = o_ext.rearrange("p (r w) -> p r w", w=OW)

            x1t = xpool.tile([128, Q * W], dt)
            x2t = xpool.tile([128, Q * W], dt)
            # interleave the two layouts so pair p's five passes unblock
            # together
            for q0 in range(0, Q, LOAD_ROWS):
                q1 = min(q0 + LOAD_ROWS, Q)
                nc.scalar.dma_start(
                    out=x1t[:, q0 * W : q1 * W], in_=x1_ext[:, q0 * W : q1 * W]
                )
                nc.scalar.dma_start(
                    out=x2t[:, q0 * W : q1 * W], in_=x2_ext[:, q0 * W : q1 * W]
                )
            x1v = x1t[:].rearrange("p (q w) -> p q w", w=W)
            x2v = x2t[:].rearrange("p (q w) -> p q w", w=W)

            for lp in range(n_pairs):
                r = 2 * lp
                ps = pspool.tile([128, 2 * OW], f32)
                # taps (0,kw)+(1,kw) for kw=0,1,2
                for kw in range(3):
                    nc.tensor.matmul(
                        ps[:],
                        lhsT=wv[:, kw, :],
                        rhs=x1v[:, r : r + 2, kw : kw + OW],
                        start=(kw == 0),
                        stop=False,
                    )
                # taps (2,0)+(2,1)
                nc.tensor.matmul(
                    ps[:],
                    lhsT=wv[:, 3, :],
                    rhs=x2v[:, r : r + 2, 0:OW],
                    start=False,
                    stop=False,
                )
                # tap (2,2) on half0 (half1 weights are zero)
                nc.tensor.matmul(
                    ps[:],
                    lhsT=wv[:, 4, :],
                    rhs=x2v[:, r : r + 2, 2 : 2 + OW],
                    start=False,
                    stop=True,
                )
                so = opool.tile([128, 2 * OW], dt)
                nc.vector.tensor_scalar_add(so[:], ps[:], bt[:, 0:1])
                nc.sync.dma_start(out=ov[:, r : r + 2, :], in_=so[:])

    _split_multi_waits(nc)
    return nc


def _split_multi_waits(nc):
    """Walrus codegen accepts a single sync-wait command per instruction.

    Tile's sem assignment happily attaches several. Hoist all but the last
    wait of every instruction onto fresh NoOps placed immediately before it
    on the same engine stream (engine streams execute in program order, so
    semantics are preserved; the wait merely moves from the instruction to
    its dispatching sequencer).
    """
    for fn in nc.m.functions:
        for bb in fn.blocks:
            out = []
            for inst in bb.instructions:
                si = inst.sync_info
                waits = list(si.on_wait) if si is not None and si.on_wait else []
                if len(waits) > 1:
                    for wt_ in waits[:-1]:
                        nop = mybir.InstNoOp(
                            name=nc.get_next_instruction_name(),
                            engine=inst.engine,
                        )
                        nop.sync_info = mybir.SyncInfo(
                            on_wait=[wt_], on_update=[]
                        )
                        nc.register_instruction(nop)
                        out.append(nop)
                    inst.sync_info = mybir.SyncInfo(
                        on_wait=[waits[-1]], on_update=list(si.on_update)
                    )
                out.append(inst)
            bb.instructions = out


def _get_program(_unused=None):
    key = "v2"
    if key not in _COMPILED:
        _COMPILED[key] = _build_program()
    return _COMPILED[key]


def _prep_inputs(x, kernels, biases, _unused=None):
    bf16 = _np_bf16()
    xp = np.zeros((IN_C, PAD_H, W), dtype=np.float32)
    xp[:, :H] = x
    xp = xp.astype(bf16)

    # wpack[:, s, :]: s=kw in 0..2 -> (kh0 on partitions 0..63, kh1 on
    # 64..127); s=3 -> (w[2,0] on 0..63, w[2,1] on 64..127); s=4 ->
    # (w[2,2] on 0..63, zeros on 64..127).
    wpack = np.zeros((128, 5, 128), dtype=np.float32)
    for kw in range(KS):
        wpack[:64, kw, :] = kernels[:, :, 0, kw].T
        wpack[64:, kw, :] = kernels[:, :, 1, kw].T
    wpack[:64, 3, :] = kernels[:, :, 2, 0].T
    wpack[64:, 3, :] = kernels[:, :, 2, 1].T
    wpack[:64, 4, :] = kernels[:, :, 2, 2].T
    wpack = wpack.reshape(128, 5 * 128).astype(bf16)

    bias = np.ascontiguousarray(biases.astype(np.float32).reshape(128, 1))

    in_maps = []
    for core in range(N_CORES):
        h0 = RPC * core
        xdup = np.empty((128, Q, W), dtype=bf16)
        xdup[:64] = xp[:, h0 : h0 + Q]
        xdup[64:] = xp[:, h0 + 1 : h0 + 1 + Q]
        xdup2 = np.zeros((128, Q, W), dtype=bf16)
        xdup2[:64] = xp[:, h0 + 2 : h0 + 2 + Q]
        xdup2[64:, :, : W - 1] = xp[:, h0 + 2 : h0 + 2 + Q, 1:]
        in_maps.append(
            {
                "xdup": xdup.reshape(128, Q * W),
                "xdup2": xdup2.reshape(128, Q * W),
                "wpack": wpack,
                "bias": bias,
            }
        )
    return in_maps


def kernel(x, kernels, biases):
    global LAST_RESULTS
    x = np.asarray(x, dtype=np.float32)
    kernels = np.asarray(kernels, dtype=np.float32)
    biases = np.asarray(biases, dtype=np.float32)

    nc = _get_program()
    in_maps = _prep_inputs(x, kernels, biases)
    res = run_bass_kernel_spmd(nc, in_maps, core_ids=list(range(N_CORES)), trace=TRACE)
    LAST_RESULTS = res

    out = np.empty((OUT_C, N_CORES * RPC, OW), dtype=np.float32)
    for c in range(N_CORES):
        out[:, RPC * c : RPC * (c + 1), :] = (
            res.results[c]["out"].astype(np.float32).reshape(OUT_C, RPC, OW)
        )
    return np.ascontiguousarray(out[:, :OH, :])


# revision 37
# speedup vs baseline: 1.4224x; 1.0561x over previous
"""Trainium2 Bass kernel for a 3x3 VALID conv: x[64,256,256] * k[128,64,3,3] -> [128,254,254].

Strategy:
  - Shard output rows across 8 cores (32 rows each; 8*32=256 >= 254, tail padded).
  - bf16 matmuls + bf16 x/w/out DMA (fp32 PSUM accumulate): rel err ~4e-3,
    half the traffic of fp32 at the same PE rate.
  - 5 accumulated matmuls per pair of output rows (the floor: each K=128
    pass contributes at most 2 of the 9 taps, ceil(9/2)=5), using two SBUF
    layouts of x:
      xdup  [128, 32*256]: partition half0 = x[q],   half1 = x[q+1]
        -> passes kw=0,1,2 each cover taps (0,kw)+(1,kw)       (3 passes)
      xdup2 [128, 32*256]: partition half0 = x[q+2], half1 = x[q+2] shifted
        one column left (pad 0)
        -> one pass covers taps (2,0)+(2,1); one half-pass covers (2,2)
    M=128 output channels, N=508 = 2 output rows x 254 cols (one PSUM bank).
  - DMA queues serialize per dispatching engine (max(bytes/partition *
    0.3855ns, 500ns) each), so loads fan out: x1 on ACT, x2 on Pool SWDGE,
    weights+bias then all steady-state stores on SP.
  - PSUM evacuation fused with the bias add (DVE for the pairs), bf16 out;
    host converts to f32.
  - The Tensor engine runs at half clock for the first 3us of the sim
    (wall-clock p-state ramp), so starting earlier always wins: a short
    dummy-matmul pad occupies the PE queue exactly until the first slices
    land, which also lets the scheduler skip the ~1.7us DMA-completion
    latency on the first real matmul.
  - Tapered tail (row30, then row31 in two chunks, evacuated on Pool and
    stored on ACT/SP) keeps the final matmul->evac->store->sem chain short.
  - Host gathers the 8 per-core output slabs.
"""

import os
import sys

import numpy as np

for _p in ("/opt/trn_rl_repo", "/root/.axon_site/_ro/trn_rl_repo"):
    if os.path.isdir(_p) and _p not in sys.path:
        sys.path.insert(0, _p)

from concourse import bass, mybir, tile  # noqa: E402
from concourse.bass_utils import run_bass_kernel_spmd  # noqa: E402

IN_C, H, W = 64, 256, 256
KS = 3
OUT_C = 128
OH, OW = H - KS + 1, W - KS + 1  # 254, 254
N_CORES = 8
RPC = 32          # output rows computed per core (8*32 = 256 >= 254)
PAD_H = 259       # padded input rows so core 7 can read h0+33+2 = 258
Q = RPC           # q-rows per layout tile
LOAD_ROWS = 4     # q-rows per load slice

def _tail31():
    n2 = int(os.environ.get("CONV_TAIL2", "64"))
    return [(0, OW - n2), (OW - n2, n2)]


N_WARM = int(os.environ.get("CONV_N_WARM", "4"))
WARM_AP = int(os.environ.get("CONV_WARM_AP", "120"))
MM_DT = "bf16"  # informational; v2 is bf16-only

# Output row groups per core: 15 row pairs (one PSUM bank each) in the
# steady state, then a tapered tail -- row30, then row31 split 190+64 --
# so the final matmul->evac->store->sem chain is as short as possible.
# Tail evacs run on the Pool engine (no PSUM-access surcharge in the cost
# model, and its x2 loads are long done); tail stores fan out over ACT/SP.
GROUPS = [(2 * i, 2, 0, OW) for i in range(15)]
TAIL31 = _tail31()  # (col0, ncols) splits of row 31

# q-row load slice boundaries: a 2-row lead slice (500ns floor) so pair 0
# unblocks early, then bulk 4-row slices.
SLICES = [(0, 2), (2, 6), (6, 10), (10, 14), (14, 18), (18, 22), (22, 26), (26, 30), (30, 32)]

TRACE = False
LAST_RESULTS = None

_COMPILED = {}

_BF16 = None


def _np_bf16():
    global _BF16
    if _BF16 is None:
        import ml_dtypes

        _BF16 = np.dtype(ml_dtypes.bfloat16)
    return _BF16


def _build_program():
    dt = mybir.dt.bfloat16
    f32 = mybir.dt.float32
    nc = bass.Bass()

    x1_ext = nc.declare_dram_parameter("xdup", [128, Q * W], dt, isOutput=False)
    x2_ext = nc.declare_dram_parameter("xdup2", [128, Q * W], dt, isOutput=False)
    w_ext = nc.declare_dram_parameter("wpack", [128, 5 * 128], dt, isOutput=False)
    b_ext = nc.declare_dram_parameter("bias", [128, 1], f32, isOutput=False)
    o_ext = nc.declare_dram_parameter("out", [128, RPC * OW], dt, isOutput=True)

    with tile.TileContext(nc) as tc:
        with (
            tc.tile_pool(name="wpool", bufs=1) as wpool,
            tc.tile_pool(name="xpool", bufs=1) as xpool,
            tc.tile_pool(name="pspool", bufs=4, space="PSUM") as pspool,
            tc.tile_pool(name="wmpool", bufs=1, space="PSUM") as wmpool,
            # bufs = n groups: output tiles are never reused -> evacuations
            # only ever wait on their PSUM producer.
            tc.tile_pool(name="opool", bufs=len(GROUPS)) as opool,
        ):
            # PE p-state warm-up: tiny memset then dummy matmuls that keep the
            # Tensor engine continuously busy while the first loads land.
            if N_WARM:
                wmt = wpool.tile([128, 128], dt)
                nc.vector.memset(wmt[:], 0.0)
                psw = wmpool.tile([128, WARM_AP], f32)
                for _ in range(N_WARM):
                    nc.tensor.matmul(
                        psw[:],
                        lhsT=wmt[:],
                        rhs=wmt[:, 0:WARM_AP],
                        start=True,
                        stop=True,
                    )

            # DMA queues are per-engine in the cost model, so fan the loads
            # out: x1 on ACT, x2 on Pool, weights+bias on SP (stores join SP
            # only after the first evac, ~4us in).
            wt = wpool.tile([128, 5 * 128], dt)
            x1t = xpool.tile([128, Q * W], dt)
            x2t = xpool.tile([128, Q * W], dt)
            bt = wpool.tile([128, 1], f32)

            nc.sync.dma_start(out=wt[:], in_=w_ext[:])
            nc.sync.dma_start(out=bt[:], in_=b_ext[:])
            for q0, q1 in SLICES:
                nc.scalar.dma_start(
                    out=x1t[:, q0 * W : q1 * W], in_=x1_ext[:, q0 * W : q1 * W]
                )
                nc.gpsimd.dma_start(
                    out=x2t[:, q0 * W : q1 * W], in_=x2_ext[:, q0 * W : q1 * W]
                )

            wv = wt[:].rearrange("p (s m) -> p s m", m=128)
            ov = o_ext.rearrange("p (r w) -> p r w", w=OW)
            x1v = x1t[:].rearrange("p (q w) -> p q w", w=W)
            x2v = x2t[:].rearrange("p (q w) -> p q w", w=W)

            def conv_passes(ps_ap, r0, nr, c0, ncol):
                psv = ps_ap.rearrange("p (b n) -> p b n", n=ncol)
                # taps (0,kw)+(1,kw) for kw=0,1,2
                for kw in range(3):
                    nc.tensor.matmul(
                        psv[:, :, :],
                        lhsT=wv[:, kw, :],
                        rhs=x1v[:, r0 : r0 + nr, c0 + kw : c0 + kw + ncol],
                        start=(kw == 0),
                        stop=False,
                    )
                # taps (2,0)+(2,1)
                nc.tensor.matmul(
                    psv[:, :, :],
                    lhsT=wv[:, 3, :],
                    rhs=x2v[:, r0 : r0 + nr, c0 : c0 + ncol],
                    start=False,
                    stop=False,
                )
                # tap (2,2) on half0 (half1 weights are zero)
                nc.tensor.matmul(
                    psv[:, :, :],
                    lhsT=wv[:, 4, :],
                    rhs=x2v[:, r0 : r0 + nr, c0 + 2 : c0 + 2 + ncol],
                    start=False,
                    stop=True,
                )

            for r0, nr, c0, ncol in GROUPS:
                ps = pspool.tile([128, nr * ncol], f32)
                conv_passes(ps[:], r0, nr, c0, ncol)
                so = opool.tile([128, nr * ncol], dt)
                nc.vector.tensor_scalar_add(so[:], ps[:], bt[:, 0:1])
                sov = so[:].rearrange("p (b n) -> p b n", n=ncol)
                nc.sync.dma_start(
                    out=ov[:, r0 : r0 + nr, c0 : c0 + ncol], in_=sov[:, :, :]
                )

            # Warm the ACT bias-add function table while ACT is otherwise
            # done (after the loads), so the tail evacs don't pay the
            # table-load latency.
            wrm = opool.tile([128, 1], dt)
            nc.scalar.add(wrm[:], wt[:, 0:1], 0.0)

            # --- tapered tail: row 30, then row 31 in two chunks ---
            # Evacs split DVE/ACT, stores split SP/Pool/ACT, so none of the
            # final chains queue behind each other.
            ps30 = pspool.tile([128, OW], f32, bufs=1)
            conv_passes(ps30[:], 30, 1, 0, OW)
            so30 = opool.tile([128, OW], dt)
            nc.scalar.add(so30[:], ps30[:], bt[:, 0:1])
            nc.gpsimd.dma_start(out=ov[:, 30, :], in_=so30[:])

            so31 = opool.tile([128, OW], dt)
            for i, (c0, ncol) in enumerate(TAIL31):
                ps31 = pspool.tile([128, ncol], f32, bufs=2)
                conv_passes(ps31[:], 31, 1, c0, ncol)
                if i == 0:
                    nc.vector.tensor_scalar_add(
                        so31[:, c0 : c0 + ncol], ps31[:], bt[:, 0:1]
                    )
                else:
                    nc.scalar.add(so31[:, c0 : c0 + ncol], ps31[:], bt[:, 0:1])
            nc.sync.dma_start(out=ov[:, 31, :], in_=so31[:])

    _split_multi_waits(nc)
    return nc


def _split_multi_waits(nc):
    """Walrus codegen accepts a single sync-wait command per instruction.

    Tile's sem assignment happily attaches several. Hoist all but the last
    wait of every instruction onto fresh NoOps placed immediately before it
    on the same engine stream (engine streams execute in program order, so
    semantics are preserved; the wait merely moves from the instruction to
    its dispatching sequencer).
    """
    for fn in nc.m.functions:
        for bb in fn.blocks:
            out = []
            for inst in bb.instructions:
                si = inst.sync_info
                waits = list(si.on_wait) if si is not None and si.on_wait else []
                if len(waits) > 1:
                    for wt_ in waits[:-1]:
                        nop = mybir.InstNoOp(
                            name=nc.get_next_instruction_name(),
                            engine=inst.engine,
                        )
                        nop.sync_info = mybir.SyncInfo(
                            on_wait=[wt_], on_update=[]
                        )
                        nc.register_instruction(nop)
                        out.append(nop)
                    inst.sync_info = mybir.SyncInfo(
                        on_wait=[waits[-1]], on_update=list(si.on_update)
                    )
                out.append(inst)
            bb.instructions = out


def _get_program(_unused=None):
    key = "v2"
    if key not in _COMPILED:
        _COMPILED[key] = _build_program()
    return _COMPILED[key]


def _prep_inputs(x, kernels, biases, _unused=None):
    bf16 = _np_bf16()
    xp = np.zeros((IN_C, PAD_H, W), dtype=np.float32)
    xp[:, :H] = x
    xp = xp.astype(bf16)

    # wpack[:, s, :]: s=kw in 0..2 -> (kh0 on partitions 0..63, kh1 on
    # 64..127); s=3 -> (w[2,0] on 0..63, w[2,1] on 64..127); s=4 ->
    # (w[2,2] on 0..63, zeros on 64..127).
    wpack = np.zeros((128, 5, 128), dtype=np.float32)
    for kw in range(KS):
        wpack[:64, kw, :] = kernels[:, :, 0, kw].T
        wpack[64:, kw, :] = kernels[:, :, 1, kw].T
    wpack[:64, 3, :] = kernels[:, :, 2, 0].T
    wpack[64:, 3, :] = kernels[:, :, 2, 1].T
    wpack[:64, 4, :] = kernels[:, :, 2, 2].T
    wpack = wpack.reshape(128, 5 * 128).astype(bf16)

    bias = np.ascontiguousarray(biases.astype(np.float32).reshape(128, 1))

    in_maps = []
    for core in range(N_CORES):
        h0 = RPC * core
        xdup = np.empty((128, Q, W), dtype=bf16)
        xdup[:64] = xp[:, h0 : h0 + Q]
        xdup[64:] = xp[:, h0 + 1 : h0 + 1 + Q]
        xdup2 = np.zeros((128, Q, W), dtype=bf16)
        xdup2[:64] = xp[:, h0 + 2 : h0 + 2 + Q]
        xdup2[64:, :, : W - 1] = xp[:, h0 + 2 : h0 + 2 + Q, 1:]
        in_maps.append(
            {
                "xdup": xdup.reshape(128, Q * W),
                "xdup2": xdup2.reshape(128, Q * W),
                "wpack": wpack,
                "bias": bias,
            }
        )
    return in_maps


def kernel(x, kernels, biases):
    global LAST_RESULTS
    x = np.asarray(x, dtype=np.float32)
    kernels = np.asarray(kernels, dtype=np.float32)
    biases = np.asarray(biases, dtype=np.float32)

    nc = _get_program()
    in_maps = _prep_inputs(x, kernels, biases)
    res = run_bass_kernel_spmd(nc, in_maps, core_ids=list(range(N_CORES)), trace=TRACE)
    LAST_RESULTS = res

    out = np.empty((OUT_C, N_CORES * RPC, OW), dtype=np.float32)
    for c in range(N_CORES):
        out[:, RPC * c : RPC * (c + 1), :] = (
            res.results[c]["out"].astype(np.float32).reshape(OUT_C, RPC, OW)
        )
    return np.ascontiguousarray(out[:, :OH, :])


# revision 45
# speedup vs baseline: 1.6068x; 1.1296x over previous
"""Trainium2 Bass kernel for a 3x3 VALID conv: x[64,256,256] * k[128,64,3,3] -> [128,254,254].

Strategy (fp8 DoubleRow with error compensation):
  - Shard output rows across 8 cores (32 rows each; 8*32=256 >= 254, tail padded).
  - Represent x ~= X8 + dX8 and 16w ~= W16 + dW16 (all fp8e4m3; the x16
    scale is a power of two so it is exact).  The three first-order terms
    X8*W16 + dX8*W16 + X8*dW16 reproduce the conv to ~1e-3 relative error
    (the dropped dX*dW term is ~1e-4); PSUM holds 16x the result and the
    evacuation rescales by 1/16 while adding the bias.
  - fp8 DoubleRow matmuls fuse TWO K=128 products per instruction and the
    cost model charges them at 0.5 cycles per output column, so the
    27 tap-terms (9 taps x 3 terms) fit in 8 DoubleRow instructions per
    output row = 8*0.5*254 cycles, vs 2.5*508 for the bf16 scheme.
  - Six precomputed fp8 x-layout "slots" live in one SBUF tile (slot-major)
    so a 3D AP [part, slot-pair, col] addresses each DoubleRow's moving
    data; per-slot partition halves carry the two packed taps:
      B: (X8[q+2]   | X8[q+2] shifted 1 col)   kernel-row-2 taps
      A: (X8[q]     | X8[q+1])                 kernel-rows-0/1 taps
      C: (dX8[q]    | dX8[q+1])
      D: (dX8[q+2]  | dX8[q+2] shifted 1 col)
      F: (X8[q] c+2 | dX8[q] c+2)              tap (0,2) for both variants
      G: (X8[q+1]c+2| dX8[q+1]c+2)             tap (1,2) for both variants
  - DMA queues serialize per dispatching engine, so loads fan out: slots
    B,A,C on SP, D,F,G on Pool, weights+bias on ACT; stores pair two rows
    and fan out over SP/Pool.
  - Evacuation = DVE tensor_scalar (x 1/16, + bias), bf16 out.
  - A short dummy-matmul pad keeps the PE queue busy until the first
    slices land (skips the ~1.7us DMA latency on the first real matmul).
  - Tapered tail: row 31 in two chunks, one merged store.
  - Host gathers the 8 per-core output slabs.
"""

import os
import sys

import numpy as np

for _p in ("/opt/trn_rl_repo", "/root/.axon_site/_ro/trn_rl_repo"):
    if os.path.isdir(_p) and _p not in sys.path:
        sys.path.insert(0, _p)

from concourse import bass, mybir, tile  # noqa: E402
from concourse.bass_utils import run_bass_kernel_spmd  # noqa: E402

IN_C, H, W = 64, 256, 256
KS = 3
OUT_C = 128
OH, OW = H - KS + 1, W - KS + 1  # 254, 254
N_CORES = 8
RPC = 32          # output rows computed per core
PAD_H = 259
Q = RPC
NSLOT = 6
# DoubleRow schedule: (weight section, first slot of the pair, column offset)
DRS = [
    (0, 0, 0),  # B,A @ +0: X8*W16 taps (2,0),(2,1),(0,0),(1,0)
    (1, 0, 1),  # B,A @ +1: X8*W16 taps (2,2),(0,1),(1,1)
    (2, 4, 0),  # F,G @ +0: X8*W16 + dX8*W16 taps (0,2),(1,2)
    (3, 2, 0),  # C,D @ +0: dX8*W16 taps (0,0),(1,0),(2,0),(2,1)
    (4, 2, 1),  # C,D @ +1: dX8*W16 taps (0,1),(1,1),(2,2)
    (5, 0, 0),  # B,A @ +0: X8*dW16 taps (2,0),(2,1),(0,0),(1,0)
    (6, 0, 1),  # B,A @ +1: X8*dW16 taps (2,2),(0,1),(1,1)
    (7, 0, 2),  # B,A @ +2: X8*dW16 taps (0,2),(1,2)
]

N_WARM = int(os.environ.get("CONV_N_WARM", "8"))
WARM_AP = int(os.environ.get("CONV_WARM_AP", "120"))
MM_DT = "fp8dr"  # informational


def _tail31():
    n2 = int(os.environ.get("CONV_TAIL2", "64"))
    return [(0, OW - n2), (OW - n2, n2)]


TAIL31 = _tail31()

# q-row load slice boundaries (per slot): 2-row lead, then 4-row bulk.
SLICES = [(0, 2), (2, 6), (6, 10), (10, 14), (14, 18), (18, 22), (22, 26), (26, 30), (30, 32)]

TRACE = False
LAST_RESULTS = None

_COMPILED = {}


def _np_dt(mdt):
    return np.dtype(mybir.dt.np(mdt))


def _np_bf16():
    return _np_dt(mybir.dt.bfloat16)


def _build_program():
    f8 = mybir.dt.float8e4
    bf = mybir.dt.bfloat16
    f32 = mybir.dt.float32
    DR = mybir.MatmulPerfMode.DoubleRow
    nc = bass.Bass()

    x_ext = nc.declare_dram_parameter("xall", [128, NSLOT * Q * W], f8, isOutput=False)
    w_ext = nc.declare_dram_parameter("wpack", [128, 8 * 2 * 128], f8, isOutput=False)
    b_ext = nc.declare_dram_parameter("bias", [128, 1], f32, isOutput=False)
    o_ext = nc.declare_dram_parameter("out", [128, RPC * OW], bf, isOutput=True)

    with tile.TileContext(nc) as tc:
        with (
            tc.tile_pool(name="wpool", bufs=1) as wpool,
            tc.tile_pool(name="xpool", bufs=1) as xpool,
            tc.tile_pool(name="pspool", bufs=4, space="PSUM") as pspool,
            tc.tile_pool(name="wmpool", bufs=1, space="PSUM") as wmpool,
            tc.tile_pool(name="opool", bufs=18) as opool,
        ):
            if N_WARM:
                wmt = wpool.tile([128, 128], bf)
                nc.vector.memset(wmt[:], 0.0)
                psw = wmpool.tile([128, WARM_AP], f32)
                for _ in range(N_WARM):
                    nc.tensor.matmul(
                        psw[:],
                        lhsT=wmt[:],
                        rhs=wmt[:, 0:WARM_AP],
                        start=True,
                        stop=True,
                    )

            wt = wpool.tile([128, 8 * 2 * 128], f8)
            xt = xpool.tile([128, NSLOT * Q * W], f8)
            bt = wpool.tile([128, 1], f32)

            # weights (2 chunks) + bias on ACT
            nc.scalar.dma_start(out=wt[:, 0 : 4 * 256], in_=w_ext[:, 0 : 4 * 256])
            nc.scalar.dma_start(out=wt[:, 4 * 256 :], in_=w_ext[:, 4 * 256 :])
            nc.scalar.dma_start(out=bt[:], in_=b_ext[:])
            # x slots: B,A,C on SP; D,F,G on Pool
            for q0, q1 in SLICES:
                for s in range(3):
                    o = s * Q * W
                    nc.sync.dma_start(
                        out=xt[:, o + q0 * W : o + q1 * W],
                        in_=x_ext[:, o + q0 * W : o + q1 * W],
                    )
                for s in range(3, 6):
                    o = s * Q * W
                    nc.gpsimd.dma_start(
                        out=xt[:, o + q0 * W : o + q1 * W],
                        in_=x_ext[:, o + q0 * W : o + q1 * W],
                    )

            wv = wt[:].rearrange("p (j t m) -> p j t m", t=2, m=128)
            ov = o_ext.rearrange("p (r w) -> p r w", w=OW)
            xv = xt[:].rearrange("p (s q w) -> p s q w", s=NSLOT, w=W)

            def conv_row(ps_ap, r0, c0, ncol):
                for i, (sec, s0, off) in enumerate(DRS):
                    nc.tensor.matmul(
                        ps_ap,
                        lhsT=wv[:, sec, :, :],
                        rhs=xv[:, s0 : s0 + 2, r0, c0 + off : c0 + off + ncol],
                        start=(i == 0),
                        stop=(i == len(DRS) - 1),
                        perf_mode=DR,
                    )

            def evac(so_ap, ps_ap):
                # out = psum/16 + bias
                nc.vector.tensor_scalar(
                    so_ap,
                    ps_ap,
                    1.0 / 16.0,
                    bt[:, 0:1],
                    mybir.AluOpType.mult,
                    mybir.AluOpType.add,
                )

            # rows 0..29 as 15 store-pairs
            for pair in range(15):
                so = opool.tile([128, 2 * OW], bf)
                for k in range(2):
                    r = 2 * pair + k
                    ps = pspool.tile([128, OW], f32)
                    conv_row(ps[:], r, 0, OW)
                    evac(so[:, k * OW : (k + 1) * OW], ps[:])
                sov = so[:].rearrange("p (b n) -> p b n", n=OW)
                # ACT only carries weights+bias, so all stores go there
                nc.scalar.dma_start(
                    out=ov[:, 2 * pair : 2 * pair + 2, :], in_=sov[:, :, :]
                )

            # tail: row30 + row31 in two chunks, one merged store on SP
            sot = opool.tile([128, 2 * OW], bf)
            ps30 = pspool.tile([128, OW], f32, bufs=1)
            conv_row(ps30[:], 30, 0, OW)
            evac(sot[:, 0:OW], ps30[:])
            for c0, ncol in TAIL31:
                ps31 = pspool.tile([128, ncol], f32, bufs=2)
                conv_row(ps31[:], 31, c0, ncol)
                evac(sot[:, OW + c0 : OW + c0 + ncol], ps31[:])
            sotv = sot[:].rearrange("p (b n) -> p b n", n=OW)
            nc.scalar.dma_start(out=ov[:, 30:32, :], in_=sotv[:, :, :])

    _split_multi_waits(nc)
    return nc


def _split_multi_waits(nc):
    """Walrus codegen accepts a single sync-wait command per instruction."""
    for fn in nc.m.functions:
        for bb in fn.blocks:
            out = []
            for inst in bb.instructions:
                si = inst.sync_info
                waits = list(si.on_wait) if si is not None and si.on_wait else []
                if len(waits) > 1:
                    for wt_ in waits[:-1]:
                        nop = mybir.InstNoOp(
                            name=nc.get_next_instruction_name(),
                            engine=inst.engine,
                        )
                        nop.sync_info = mybir.SyncInfo(on_wait=[wt_], on_update=[])
                        nc.register_instruction(nop)
                        out.append(nop)
                    inst.sync_info = mybir.SyncInfo(
                        on_wait=[waits[-1]], on_update=list(si.on_update)
                    )
                out.append(inst)
            bb.instructions = out


def _get_program(_unused=None):
    key = "v8"
    if key not in _COMPILED:
        _COMPILED[key] = _build_program()
    return _COMPILED[key]


def _prep_inputs(x, kernels, biases, _unused=None):
    f8 = _np_dt(mybir.dt.float8e4)
    bf16 = _np_dt(mybir.dt.bfloat16)

    xp = np.zeros((IN_C, PAD_H, W), dtype=np.float32)
    xp[:, :H] = x
    X8f = xp.astype(f8)
    X8 = X8f.astype(np.float32)
    dX8f = (xp - X8).astype(f8)

    w16 = kernels.astype(np.float32) * 16.0
    W16f = w16.astype(f8)
    W16 = W16f.astype(np.float32)
    dW16f = (w16 - W16).astype(f8)
    W16 = W16f.astype(np.float32)

    def wsec(wf, kh, kw):
        # [64, 128] fp8->f32 weight block transposed (chan, outch)
        return wf[:, :, kh, kw].T.astype(np.float32)

    # 8 sections x 2 slots x [128 part, 128 outch]
    wpack = np.zeros((128, 8, 2, 128), dtype=np.float32)

    def fill(sec, t, upper, lower):
        if upper is not None:
            wpack[:64, sec, t, :] = upper
        if lower is not None:
            wpack[64:, sec, t, :] = lower

    Wf, dWf = W16f, dW16f
    fill(0, 0, wsec(Wf, 2, 0), wsec(Wf, 2, 1))
    fill(0, 1, wsec(Wf, 0, 0), wsec(Wf, 1, 0))
    fill(1, 0, None, wsec(Wf, 2, 2))
    fill(1, 1, wsec(Wf, 0, 1), wsec(Wf, 1, 1))
    fill(2, 0, wsec(Wf, 0, 2), wsec(Wf, 0, 2))
    fill(2, 1, wsec(Wf, 1, 2), wsec(Wf, 1, 2))
    fill(3, 0, wsec(Wf, 0, 0), wsec(Wf, 1, 0))
    fill(3, 1, wsec(Wf, 2, 0), wsec(Wf, 2, 1))
    fill(4, 0, wsec(Wf, 0, 1), wsec(Wf, 1, 1))
    fill(4, 1, None, wsec(Wf, 2, 2))
    fill(5, 0, wsec(dWf, 2, 0), wsec(dWf, 2, 1))
    fill(5, 1, wsec(dWf, 0, 0), wsec(dWf, 1, 0))
    fill(6, 0, None, wsec(dWf, 2, 2))
    fill(6, 1, wsec(dWf, 0, 1), wsec(dWf, 1, 1))
    fill(7, 1, wsec(dWf, 0, 2), wsec(dWf, 1, 2))
    wpack = wpack.reshape(128, 8 * 2 * 128).astype(f8)

    bias = np.ascontiguousarray(biases.astype(np.float32).reshape(128, 1))

    in_maps = []
    for core in range(N_CORES):
        h0 = RPC * core
        xs = np.zeros((128, NSLOT, Q, W), dtype=f8)

        def seg(src, row0, colshift=0):
            # [64, Q, W] slice of src at rows row0.., columns shifted left
            out = np.zeros((64, Q, W), dtype=src.dtype)
            if colshift == 0:
                out[:] = src[:, row0 : row0 + Q]
            else:
                out[:, :, : W - colshift] = src[:, row0 : row0 + Q, colshift:]
            return out

        xs[:64, 0] = seg(X8f, h0 + 2)            # B upper
        xs[64:, 0] = seg(X8f, h0 + 2, 1)         # B lower
        xs[:64, 1] = seg(X8f, h0)                # A upper
        xs[64:, 1] = seg(X8f, h0 + 1)            # A lower
        xs[:64, 2] = seg(dX8f, h0)               # C upper
        xs[64:, 2] = seg(dX8f, h0 + 1)           # C lower
        xs[:64, 3] = seg(dX8f, h0 + 2)           # D upper
        xs[64:, 3] = seg(dX8f, h0 + 2, 1)        # D lower
        xs[:64, 4] = seg(X8f, h0, 2)             # F upper
        xs[64:, 4] = seg(dX8f, h0, 2)            # F lower
        xs[:64, 5] = seg(X8f, h0 + 1, 2)         # G upper
        xs[64:, 5] = seg(dX8f, h0 + 1, 2)        # G lower

        in_maps.append(
            {
                "xall": xs.reshape(128, NSLOT * Q * W),
                "wpack": wpack,
                "bias": bias,
            }
        )
    return in_maps


def kernel(x, kernels, biases):
    global LAST_RESULTS
    x = np.asarray(x, dtype=np.float32)
    kernels = np.asarray(kernels, dtype=np.float32)
    biases = np.asarray(biases, dtype=np.float32)

    nc = _get_program()
    in_maps = _prep_inputs(x, kernels, biases)
    res = run_bass_kernel_spmd(nc, in_maps, core_ids=list(range(N_CORES)), trace=TRACE)
    LAST_RESULTS = res

    out = np.empty((OUT_C, N_CORES * RPC, OW), dtype=np.float32)
    for c in range(N_CORES):
        out[:, RPC * c : RPC * (c + 1), :] = (
            res.results[c]["out"].astype(np.float32).reshape(OUT_C, RPC, OW)
        )
    return np.ascontiguousarray(out[:, :OH, :])


# revision 50
# speedup vs baseline: 1.6795x; 1.0452x over previous
"""Trainium2 Bass kernel for a 3x3 VALID conv: x[64,256,256] * k[128,64,3,3] -> [128,254,254].

Strategy (fp8 DoubleRow with error compensation):
  - Shard output rows across 8 cores (32 rows each; 8*32=256 >= 254, tail padded).
  - Represent x ~= X8 + dX8 and 16w ~= W16 + dW16 (all fp8e4m3; the x16
    scale is a power of two so it is exact).  The three first-order terms
    X8*W16 + dX8*W16 + X8*dW16 reproduce the conv to ~1e-3 relative error
    (the dropped dX*dW term is ~1e-4); PSUM holds 16x the result and the
    evacuation rescales by 1/16 while adding the bias.
  - fp8 DoubleRow matmuls fuse TWO K=128 products per instruction and the
    cost model charges them at 0.5 cycles per output column, so the
    27 tap-terms (9 taps x 3 terms) fit in 8 DoubleRow instructions per
    output row = 8*0.5*254 cycles, vs 2.5*508 for the bf16 scheme.
  - Six precomputed fp8 x-layout "slots" live in one SBUF tile (slot-major)
    so a 3D AP [part, slot-pair, col] addresses each DoubleRow's moving
    data; per-slot partition halves carry the two packed taps:
      B: (X8[q+2]   | X8[q+2] shifted 1 col)   kernel-row-2 taps
      A: (X8[q]     | X8[q+1])                 kernel-rows-0/1 taps
      C: (dX8[q]    | dX8[q+1])
      D: (dX8[q+2]  | dX8[q+2] shifted 1 col)
      F: (X8[q] c+2 | dX8[q] c+2)              tap (0,2) for both variants
      G: (X8[q+1]c+2| dX8[q+1]c+2)             tap (1,2) for both variants
  - DMA queues serialize per dispatching engine, so loads fan out: slots
    B,A,C on SP, D,F,G on Pool, weights+bias on ACT; stores pair two rows
    and fan out over SP/Pool.
  - Evacuation = DVE tensor_scalar (x 1/16, + bias), bf16 out.
  - A short dummy-matmul pad keeps the PE queue busy until the first
    slices land (skips the ~1.7us DMA latency on the first real matmul).
  - Tapered tail: row 31 in two chunks, one merged store.
  - Host gathers the 8 per-core output slabs.
"""

import os
import sys

import numpy as np

for _p in ("/opt/trn_rl_repo", "/root/.axon_site/_ro/trn_rl_repo"):
    if os.path.isdir(_p) and _p not in sys.path:
        sys.path.insert(0, _p)

from concourse import bass, mybir, tile  # noqa: E402
from concourse.bass_utils import run_bass_kernel_spmd  # noqa: E402

IN_C, H, W = 64, 256, 256
KS = 3
OUT_C = 128
OH, OW = H - KS + 1, W - KS + 1  # 254, 254
N_CORES = 8
RPC = 32          # output rows computed per core
PAD_H = 259
Q = RPC
NSLOT = 6
# DoubleRow schedule: (weight section, first slot of the pair, column offset)
DRS = [
    (0, 0, 0),  # B,A @ +0: X8*W16 taps (2,0),(2,1),(0,0),(1,0)
    (1, 0, 1),  # B,A @ +1: X8*W16 taps (2,2),(0,1),(1,1)
    (5, 0, 0),  # B,A @ +0: X8*dW16 taps (2,0),(2,1),(0,0),(1,0)
    (6, 0, 1),  # B,A @ +1: X8*dW16 taps (2,2),(0,1),(1,1)
    (7, 0, 2),  # B,A @ +2: X8*dW16 taps (0,2),(1,2)
    (3, 2, 0),  # C,D @ +0: dX8*W16 taps (0,0),(1,0),(2,0),(2,1)
    (4, 2, 1),  # C,D @ +1: dX8*W16 taps (0,1),(1,1),(2,2)
    (2, 4, 0),  # F,G @ +0: X8*W16 + dX8*W16 taps (0,2),(1,2)
]

N_WARM = int(os.environ.get("CONV_N_WARM", "3"))
WARM_AP = int(os.environ.get("CONV_WARM_AP", "120"))
MM_DT = "fp8dr"  # informational


def _tail31():
    n2 = int(os.environ.get("CONV_TAIL2", "64"))
    return [(0, OW - n2), (OW - n2, n2)]


TAIL31 = _tail31()

# q-row load slice boundaries (per slot): 2-row lead, then 4-row bulk.
SLICES = [(0, 2), (2, 6), (6, 10), (10, 14), (14, 18), (18, 22), (22, 26), (26, 30), (30, 32)]

TRACE = False
LAST_RESULTS = None

_COMPILED = {}


def _np_dt(mdt):
    return np.dtype(mybir.dt.np(mdt))


def _np_bf16():
    return _np_dt(mybir.dt.bfloat16)


def _build_program():
    f8 = mybir.dt.float8e4
    bf = mybir.dt.bfloat16
    f32 = mybir.dt.float32
    DR = mybir.MatmulPerfMode.DoubleRow
    nc = bass.Bass()

    x_ext = nc.declare_dram_parameter("xall", [128, NSLOT * Q * W], f8, isOutput=False)
    w_ext = nc.declare_dram_parameter("wpack", [128, 8 * 2 * 128], f8, isOutput=False)
    b_ext = nc.declare_dram_parameter("bias", [128, 1], f32, isOutput=False)
    o_ext = nc.declare_dram_parameter("out", [128, RPC * OW], bf, isOutput=True)

    with tile.TileContext(nc) as tc:
        with (
            tc.tile_pool(name="wpool", bufs=1) as wpool,
            tc.tile_pool(name="xpool", bufs=1) as xpool,
            tc.tile_pool(name="pspool", bufs=4, space="PSUM") as pspool,
            tc.tile_pool(name="wmpool", bufs=1, space="PSUM") as wmpool,
            tc.tile_pool(name="opool", bufs=18) as opool,
        ):
            if N_WARM:
                wmt = wpool.tile([128, 128], bf)
                nc.vector.memset(wmt[:], 0.0)
                psw = wmpool.tile([128, WARM_AP], f32)
                for _ in range(N_WARM):
                    nc.tensor.matmul(
                        psw[:],
                        lhsT=wmt[:],
                        rhs=wmt[:, 0:WARM_AP],
                        start=True,
                        stop=True,
                    )

            wt = wpool.tile([128, 8 * 2 * 128], f8)
            xt = xpool.tile([128, NSLOT * Q * W], f8)
            bt = wpool.tile([128, 1], f32)

            # weights (2 chunks) + bias on ACT
            nc.scalar.dma_start(out=wt[:, 0 : 4 * 256], in_=w_ext[:, 0 : 4 * 256])
            nc.scalar.dma_start(out=wt[:, 4 * 256 :], in_=w_ext[:, 4 * 256 :])
            nc.scalar.dma_start(out=bt[:], in_=b_ext[:])
            wrm = wpool.tile([128, 1], mybir.dt.bfloat16)
            nc.scalar.activation(
                wrm[:], bt[:, 0:1], mybir.ActivationFunctionType.Identity
            )
            # x slots: each DoubleRow slot-pair is split across SP and
            # Pool so both halves of a pair arrive in parallel; order
            # matches the DR schedule (B,A first, then C,D, then F,G).
            for q0, q1 in SLICES:
                for s in (1, 2, 4):  # A, C, F on SP
                    o = s * Q * W
                    nc.sync.dma_start(
                        out=xt[:, o + q0 * W : o + q1 * W],
                        in_=x_ext[:, o + q0 * W : o + q1 * W],
                    )
                for s in (0, 3, 5):  # B, D, G on Pool
                    o = s * Q * W
                    nc.gpsimd.dma_start(
                        out=xt[:, o + q0 * W : o + q1 * W],
                        in_=x_ext[:, o + q0 * W : o + q1 * W],
                    )

            wv = wt[:].rearrange("p (j t m) -> p j t m", t=2, m=128)
            ov = o_ext.rearrange("p (r w) -> p r w", w=OW)
            xv = xt[:].rearrange("p (s q w) -> p s q w", s=NSLOT, w=W)

            def conv_row(ps_ap, r0, c0, ncol):
                for i, (sec, s0, off) in enumerate(DRS):
                    nc.tensor.matmul(
                        ps_ap,
                        lhsT=wv[:, sec, :, :],
                        rhs=xv[:, s0 : s0 + 2, r0, c0 + off : c0 + off + ncol],
                        start=(i == 0),
                        stop=(i == len(DRS) - 1),
                        perf_mode=DR,
                    )

            def evac(so_ap, ps_ap):
                # out = psum/16 + bias
                nc.vector.tensor_scalar(
                    so_ap,
                    ps_ap,
                    1.0 / 16.0,
                    bt[:, 0:1],
                    mybir.AluOpType.mult,
                    mybir.AluOpType.add,
                )

            # rows 0..29 as 15 store-pairs
            for pair in range(15):
                so = opool.tile([128, 2 * OW], bf)
                for k in range(2):
                    r = 2 * pair + k
                    ps = pspool.tile([128, OW], f32)
                    conv_row(ps[:], r, 0, OW)
                    evac(so[:, k * OW : (k + 1) * OW], ps[:])
                sov = so[:].rearrange("p (b n) -> p b n", n=OW)
                # early stores on ACT (idle but for weights); late ones on
                # SP (idle after its slot loads) so ACT is free for the
                # tail evacuations
                st = nc.scalar if pair < 12 else nc.sync
                st.dma_start(
                    out=ov[:, 2 * pair : 2 * pair + 2, :], in_=sov[:, :, :]
                )

            # tail: row30 + row31 in two chunks, one merged store on SP
            def act_evac(so_ap, ps_ap):
                nc.scalar.activation(
                    so_ap,
                    ps_ap,
                    mybir.ActivationFunctionType.Identity,
                    bias=bt[:, 0:1],
                    scale=1.0 / 16.0,
                )

            sot = opool.tile([128, 2 * OW], bf)
            ps30 = pspool.tile([128, OW], f32, bufs=1)
            conv_row(ps30[:], 30, 0, OW)
            act_evac(sot[:, 0:OW], ps30[:])
            for i, (c0, ncol) in enumerate(TAIL31):
                ps31 = pspool.tile([128, ncol], f32, bufs=2)
                conv_row(ps31[:], 31, c0, ncol)
                if i == 0:
                    evac(sot[:, OW + c0 : OW + c0 + ncol], ps31[:])
                else:
                    act_evac(sot[:, OW + c0 : OW + c0 + ncol], ps31[:])
            sotv = sot[:].rearrange("p (b n) -> p b n", n=OW)
            nc.sync.dma_start(out=ov[:, 30:32, :], in_=sotv[:, :, :])

    _split_multi_waits(nc)
    return nc


def _split_multi_waits(nc):
    """Walrus codegen accepts a single sync-wait command per instruction."""
    for fn in nc.m.functions:
        for bb in fn.blocks:
            out = []
            for inst in bb.instructions:
                si = inst.sync_info
                waits = list(si.on_wait) if si is not None and si.on_wait else []
                if len(waits) > 1:
                    for wt_ in waits[:-1]:
                        nop = mybir.InstNoOp(
                            name=nc.get_next_instruction_name(),
                            engine=inst.engine,
                        )
                        nop.sync_info = mybir.SyncInfo(on_wait=[wt_], on_update=[])
                        nc.register_instruction(nop)
                        out.append(nop)
                    inst.sync_info = mybir.SyncInfo(
                        on_wait=[waits[-1]], on_update=list(si.on_update)
                    )
                out.append(inst)
            bb.instructions = out


def _get_program(_unused=None):
    key = "v8"
    if key not in _COMPILED:
        _COMPILED[key] = _build_program()
    return _COMPILED[key]


def _prep_inputs(x, kernels, biases, _unused=None):
    f8 = _np_dt(mybir.dt.float8e4)
    bf16 = _np_dt(mybir.dt.bfloat16)

    xp = np.zeros((IN_C, PAD_H, W), dtype=np.float32)
    xp[:, :H] = x
    X8f = xp.astype(f8)
    X8 = X8f.astype(np.float32)
    dX8f = (xp - X8).astype(f8)

    w16 = kernels.astype(np.float32) * 16.0
    W16f = w16.astype(f8)
    W16 = W16f.astype(np.float32)
    dW16f = (w16 - W16).astype(f8)
    W16 = W16f.astype(np.float32)

    def wsec(wf, kh, kw):
        # [64, 128] fp8->f32 weight block transposed (chan, outch)
        return wf[:, :, kh, kw].T.astype(np.float32)

    # 8 sections x 2 slots x [128 part, 128 outch]
    wpack = np.zeros((128, 8, 2, 128), dtype=np.float32)

    def fill(sec, t, upper, lower):
        if upper is not None:
            wpack[:64, sec, t, :] = upper
        if lower is not None:
            wpack[64:, sec, t, :] = lower

    Wf, dWf = W16f, dW16f
    fill(0, 0, wsec(Wf, 2, 0), wsec(Wf, 2, 1))
    fill(0, 1, wsec(Wf, 0, 0), wsec(Wf, 1, 0))
    fill(1, 0, None, wsec(Wf, 2, 2))
    fill(1, 1, wsec(Wf, 0, 1), wsec(Wf, 1, 1))
    fill(2, 0, wsec(Wf, 0, 2), wsec(Wf, 0, 2))
    fill(2, 1, wsec(Wf, 1, 2), wsec(Wf, 1, 2))
    fill(3, 0, wsec(Wf, 0, 0), wsec(Wf, 1, 0))
    fill(3, 1, wsec(Wf, 2, 0), wsec(Wf, 2, 1))
    fill(4, 0, wsec(Wf, 0, 1), wsec(Wf, 1, 1))
    fill(4, 1, None, wsec(Wf, 2, 2))
    fill(5, 0, wsec(dWf, 2, 0), wsec(dWf, 2, 1))
    fill(5, 1, wsec(dWf, 0, 0), wsec(dWf, 1, 0))
    fill(6, 0, None, wsec(dWf, 2, 2))
    fill(6, 1, wsec(dWf, 0, 1), wsec(dWf, 1, 1))
    fill(7, 1, wsec(dWf, 0, 2), wsec(dWf, 1, 2))
    wpack = wpack.reshape(128, 8 * 2 * 128).astype(f8)

    bias = np.ascontiguousarray(biases.astype(np.float32).reshape(128, 1))

    in_maps = []
    for core in range(N_CORES):
        h0 = RPC * core
        xs = np.zeros((128, NSLOT, Q, W), dtype=f8)

        def seg(src, row0, colshift=0):
            # [64, Q, W] slice of src at rows row0.., columns shifted left
            out = np.zeros((64, Q, W), dtype=src.dtype)
            if colshift == 0:
                out[:] = src[:, row0 : row0 + Q]
            else:
                out[:, :, : W - colshift] = src[:, row0 : row0 + Q, colshift:]
            return out

        xs[:64, 0] = seg(X8f, h0 + 2)            # B upper
        xs[64:, 0] = seg(X8f, h0 + 2, 1)         # B lower
        xs[:64, 1] = seg(X8f, h0)                # A upper
        xs[64:, 1] = seg(X8f, h0 + 1)            # A lower
        xs[:64, 2] = seg(dX8f, h0)               # C upper
        xs[64:, 2] = seg(dX8f, h0 + 1)           # C lower
        xs[:64, 3] = seg(dX8f, h0 + 2)           # D upper
        xs[64:, 3] = seg(dX8f, h0 + 2, 1)        # D lower
        xs[:64, 4] = seg(X8f, h0, 2)             # F upper
        xs[64:, 4] = seg(dX8f, h0, 2)            # F lower
        xs[:64, 5] = seg(X8f, h0 + 1, 2)         # G upper
        xs[64:, 5] = seg(dX8f, h0 + 1, 2)        # G lower

        in_maps.append(
            {
                "xall": xs.reshape(128, NSLOT * Q * W),
                "wpack": wpack,
                "bias": bias,
            }
        )
    return in_maps


def kernel(x, kernels, biases):
    global LAST_RESULTS
    x = np.asarray(x, dtype=np.float32)
    kernels = np.asarray(kernels, dtype=np.float32)
    biases = np.asarray(biases, dtype=np.float32)

    nc = _get_program()
    in_maps = _prep_inputs(x, kernels, biases)
    res = run_bass_kernel_spmd(nc, in_maps, core_ids=list(range(N_CORES)), trace=TRACE)
    LAST_RESULTS = res

    out = np.empty((OUT_C, N_CORES * RPC, OW), dtype=np.float32)
    for c in range(N_CORES):
        out[:, RPC * c : RPC * (c + 1), :] = (
            res.results[c]["out"].astype(np.float32).reshape(OUT_C, RPC, OW)
        )
    return np.ascontiguousarray(out[:, :OH, :])


# revision 51
# speedup vs baseline: 1.6818x; 1.0014x over previous
"""Trainium2 Bass kernel for a 3x3 VALID conv: x[64,256,256] * k[128,64,3,3] -> [128,254,254].

Strategy (fp8 DoubleRow with error compensation):
  - Shard output rows across 8 cores (32 rows each; 8*32=256 >= 254, tail padded).
  - Represent x ~= X8 + dX8 and 16w ~= W16 + dW16 (all fp8e4m3; the x16
    scale is a power of two so it is exact).  The three first-order terms
    X8*W16 + dX8*W16 + X8*dW16 reproduce the conv to ~1e-3 relative error
    (the dropped dX*dW term is ~1e-4); PSUM holds 16x the result and the
    evacuation rescales by 1/16 while adding the bias.
  - fp8 DoubleRow matmuls fuse TWO K=128 products per instruction and the
    cost model charges them at 0.5 cycles per output column, so the
    27 tap-terms (9 taps x 3 terms) fit in 8 DoubleRow instructions per
    output row = 8*0.5*254 cycles, vs 2.5*508 for the bf16 scheme.
  - Six precomputed fp8 x-layout "slots" live in one SBUF tile (slot-major)
    so a 3D AP [part, slot-pair, col] addresses each DoubleRow's moving
    data; per-slot partition halves carry the two packed taps:
      B: (X8[q+2]   | X8[q+2] shifted 1 col)   kernel-row-2 taps
      A: (X8[q]     | X8[q+1])                 kernel-rows-0/1 taps
      C: (dX8[q]    | dX8[q+1])
      D: (dX8[q+2]  | dX8[q+2] shifted 1 col)
      F: (X8[q] c+2 | dX8[q] c+2)              tap (0,2) for both variants
      G: (X8[q+1]c+2| dX8[q+1]c+2)             tap (1,2) for both variants
  - DMA queues serialize per dispatching engine, so loads fan out: slots
    B,A,C on SP, D,F,G on Pool, weights+bias on ACT; stores pair two rows
    and fan out over SP/Pool.
  - Evacuation = DVE tensor_scalar (x 1/16, + bias), bf16 out.
  - A short dummy-matmul pad keeps the PE queue busy until the first
    slices land (skips the ~1.7us DMA latency on the first real matmul).
  - Tapered tail: row 31 in two chunks, one merged store.
  - Host gathers the 8 per-core output slabs.
"""

import os
import sys

import numpy as np

for _p in ("/opt/trn_rl_repo", "/root/.axon_site/_ro/trn_rl_repo"):
    if os.path.isdir(_p) and _p not in sys.path:
        sys.path.insert(0, _p)

from concourse import bass, mybir, tile  # noqa: E402
from concourse.bass_utils import run_bass_kernel_spmd  # noqa: E402

IN_C, H, W = 64, 256, 256
KS = 3
OUT_C = 128
OH, OW = H - KS + 1, W - KS + 1  # 254, 254
N_CORES = 8
RPC = 32          # output rows computed per core
PAD_H = 259
Q = RPC
NSLOT = 6
# DoubleRow schedule: (weight section, first slot of the pair, column offset)
DRS = [
    (0, 0, 0),  # B,A @ +0: X8*W16 taps (2,0),(2,1),(0,0),(1,0)
    (1, 0, 1),  # B,A @ +1: X8*W16 taps (2,2),(0,1),(1,1)
    (5, 0, 0),  # B,A @ +0: X8*dW16 taps (2,0),(2,1),(0,0),(1,0)
    (6, 0, 1),  # B,A @ +1: X8*dW16 taps (2,2),(0,1),(1,1)
    (7, 0, 2),  # B,A @ +2: X8*dW16 taps (0,2),(1,2)
    (3, 2, 0),  # C,D @ +0: dX8*W16 taps (0,0),(1,0),(2,0),(2,1)
    (4, 2, 1),  # C,D @ +1: dX8*W16 taps (0,1),(1,1),(2,2)
    (2, 4, 0),  # F,G @ +0: X8*W16 + dX8*W16 taps (0,2),(1,2)
]

N_WARM = int(os.environ.get("CONV_N_WARM", "3"))
WARM_AP = int(os.environ.get("CONV_WARM_AP", "120"))
MM_DT = "fp8dr"  # informational


def _tail31():
    n2 = int(os.environ.get("CONV_TAIL2", "32"))
    return [(0, OW - n2), (OW - n2, n2)]


TAIL31 = _tail31()

# q-row load slice boundaries (per slot): 2-row lead, then 4-row bulk.
SLICES = [(0, 2), (2, 6), (6, 10), (10, 14), (14, 18), (18, 22), (22, 26), (26, 30), (30, 32)]

TRACE = False
LAST_RESULTS = None

_COMPILED = {}


def _np_dt(mdt):
    return np.dtype(mybir.dt.np(mdt))


def _np_bf16():
    return _np_dt(mybir.dt.bfloat16)


def _build_program():
    f8 = mybir.dt.float8e4
    bf = mybir.dt.bfloat16
    f32 = mybir.dt.float32
    DR = mybir.MatmulPerfMode.DoubleRow
    nc = bass.Bass()

    x_ext = nc.declare_dram_parameter("xall", [128, NSLOT * Q * W], f8, isOutput=False)
    w_ext = nc.declare_dram_parameter("wpack", [128, 8 * 2 * 128], f8, isOutput=False)
    b_ext = nc.declare_dram_parameter("bias", [128, 1], f32, isOutput=False)
    o_ext = nc.declare_dram_parameter("out", [128, RPC * OW], bf, isOutput=True)

    with tile.TileContext(nc) as tc:
        with (
            tc.tile_pool(name="wpool", bufs=1) as wpool,
            tc.tile_pool(name="xpool", bufs=1) as xpool,
            tc.tile_pool(name="pspool", bufs=4, space="PSUM") as pspool,
            tc.tile_pool(name="wmpool", bufs=1, space="PSUM") as wmpool,
            tc.tile_pool(name="opool", bufs=18) as opool,
        ):
            if N_WARM:
                wmt = wpool.tile([128, 128], bf)
                nc.vector.memset(wmt[:], 0.0)
                psw = wmpool.tile([128, WARM_AP], f32)
                for _ in range(N_WARM):
                    nc.tensor.matmul(
                        psw[:],
                        lhsT=wmt[:],
                        rhs=wmt[:, 0:WARM_AP],
                        start=True,
                        stop=True,
                    )

            wt = wpool.tile([128, 8 * 2 * 128], f8)
            xt = xpool.tile([128, NSLOT * Q * W], f8)
            bt = wpool.tile([128, 1], f32)

            # weights (2 chunks) + bias on ACT
            nc.scalar.dma_start(out=wt[:, 0 : 4 * 256], in_=w_ext[:, 0 : 4 * 256])
            nc.scalar.dma_start(out=wt[:, 4 * 256 :], in_=w_ext[:, 4 * 256 :])
            nc.scalar.dma_start(out=bt[:], in_=b_ext[:])
            wrm = wpool.tile([128, 1], mybir.dt.bfloat16)
            nc.scalar.activation(
                wrm[:], bt[:, 0:1], mybir.ActivationFunctionType.Identity
            )
            # x slots: each DoubleRow slot-pair is split across SP and
            # Pool so both halves of a pair arrive in parallel; order
            # matches the DR schedule (B,A first, then C,D, then F,G).
            for q0, q1 in SLICES:
                for s in (1, 2, 4):  # A, C, F on SP
                    o = s * Q * W
                    nc.sync.dma_start(
                        out=xt[:, o + q0 * W : o + q1 * W],
                        in_=x_ext[:, o + q0 * W : o + q1 * W],
                    )
                for s in (0, 3, 5):  # B, D, G on Pool
                    o = s * Q * W
                    nc.gpsimd.dma_start(
                        out=xt[:, o + q0 * W : o + q1 * W],
                        in_=x_ext[:, o + q0 * W : o + q1 * W],
                    )

            wv = wt[:].rearrange("p (j t m) -> p j t m", t=2, m=128)
            ov = o_ext.rearrange("p (r w) -> p r w", w=OW)
            xv = xt[:].rearrange("p (s q w) -> p s q w", s=NSLOT, w=W)

            def conv_row(ps_ap, r0, c0, ncol):
                for i, (sec, s0, off) in enumerate(DRS):
                    nc.tensor.matmul(
                        ps_ap,
                        lhsT=wv[:, sec, :, :],
                        rhs=xv[:, s0 : s0 + 2, r0, c0 + off : c0 + off + ncol],
                        start=(i == 0),
                        stop=(i == len(DRS) - 1),
                        perf_mode=DR,
                    )

            def evac(so_ap, ps_ap):
                # out = psum/16 + bias
                nc.vector.tensor_scalar(
                    so_ap,
                    ps_ap,
                    1.0 / 16.0,
                    bt[:, 0:1],
                    mybir.AluOpType.mult,
                    mybir.AluOpType.add,
                )

            # rows 0..29 as 15 store-pairs
            for pair in range(15):
                so = opool.tile([128, 2 * OW], bf)
                for k in range(2):
                    r = 2 * pair + k
                    ps = pspool.tile([128, OW], f32)
                    conv_row(ps[:], r, 0, OW)
                    evac(so[:, k * OW : (k + 1) * OW], ps[:])
                sov = so[:].rearrange("p (b n) -> p b n", n=OW)
                # early stores on ACT (idle but for weights); late ones on
                # SP (idle after its slot loads) so ACT is free for the
                # tail evacuations
                st = nc.scalar if pair < 12 else nc.sync
                st.dma_start(
                    out=ov[:, 2 * pair : 2 * pair + 2, :], in_=sov[:, :, :]
                )

            # tail: row30 + row31 in two chunks, one merged store on SP
            def act_evac(so_ap, ps_ap):
                nc.scalar.activation(
                    so_ap,
                    ps_ap,
                    mybir.ActivationFunctionType.Identity,
                    bias=bt[:, 0:1],
                    scale=1.0 / 16.0,
                )

            sot = opool.tile([128, 2 * OW], bf)
            ps30 = pspool.tile([128, OW], f32, bufs=1)
            conv_row(ps30[:], 30, 0, OW)
            act_evac(sot[:, 0:OW], ps30[:])
            for i, (c0, ncol) in enumerate(TAIL31):
                ps31 = pspool.tile([128, ncol], f32, bufs=2)
                conv_row(ps31[:], 31, c0, ncol)
                if i == 0:
                    evac(sot[:, OW + c0 : OW + c0 + ncol], ps31[:])
                else:
                    act_evac(sot[:, OW + c0 : OW + c0 + ncol], ps31[:])
            sotv = sot[:].rearrange("p (b n) -> p b n", n=OW)
            nc.sync.dma_start(out=ov[:, 30:32, :], in_=sotv[:, :, :])

    _split_multi_waits(nc)
    return nc


def _split_multi_waits(nc):
    """Walrus codegen accepts a single sync-wait command per instruction."""
    for fn in nc.m.functions:
        for bb in fn.blocks:
            out = []
            for inst in bb.instructions:
                si = inst.sync_info
                waits = list(si.on_wait) if si is not None and si.on_wait else []
                if len(waits) > 1:
                    for wt_ in waits[:-1]:
                        nop = mybir.InstNoOp(
                            name=nc.get_next_instruction_name(),
                            engine=inst.engine,
                        )
                        nop.sync_info = mybir.SyncInfo(on_wait=[wt_], on_update=[])
                        nc.register_instruction(nop)
                        out.append(nop)
                    inst.sync_info = mybir.SyncInfo(
                        on_wait=[waits[-1]], on_update=list(si.on_update)
                    )
                out.append(inst)
            bb.instructions = out


def _get_program(_unused=None):
    key = "v8"
    if key not in _COMPILED:
        _COMPILED[key] = _build_program()
    return _COMPILED[key]


def _prep_inputs(x, kernels, biases, _unused=None):
    f8 = _np_dt(mybir.dt.float8e4)
    bf16 = _np_dt(mybir.dt.bfloat16)

    xp = np.zeros((IN_C, PAD_H, W), dtype=np.float32)
    xp[:, :H] = x
    X8f = xp.astype(f8)
    X8 = X8f.astype(np.float32)
    dX8f = (xp - X8).astype(f8)

    w16 = kernels.astype(np.float32) * 16.0
    W16f = w16.astype(f8)
    W16 = W16f.astype(np.float32)
    dW16f = (w16 - W16).astype(f8)
    W16 = W16f.astype(np.float32)

    def wsec(wf, kh, kw):
        # [64, 128] fp8->f32 weight block transposed (chan, outch)
        return wf[:, :, kh, kw].T.astype(np.float32)

    # 8 sections x 2 slots x [128 part, 128 outch]
    wpack = np.zeros((128, 8, 2, 128), dtype=np.float32)

    def fill(sec, t, upper, lower):
        if upper is not None:
            wpack[:64, sec, t, :] = upper
        if lower is not None:
            wpack[64:, sec, t, :] = lower

    Wf, dWf = W16f, dW16f
    fill(0, 0, wsec(Wf, 2, 0), wsec(Wf, 2, 1))
    fill(0, 1, wsec(Wf, 0, 0), wsec(Wf, 1, 0))
    fill(1, 0, None, wsec(Wf, 2, 2))
    fill(1, 1, wsec(Wf, 0, 1), wsec(Wf, 1, 1))
    fill(2, 0, wsec(Wf, 0, 2), wsec(Wf, 0, 2))
    fill(2, 1, wsec(Wf, 1, 2), wsec(Wf, 1, 2))
    fill(3, 0, wsec(Wf, 0, 0), wsec(Wf, 1, 0))
    fill(3, 1, wsec(Wf, 2, 0), wsec(Wf, 2, 1))
    fill(4, 0, wsec(Wf, 0, 1), wsec(Wf, 1, 1))
    fill(4, 1, None, wsec(Wf, 2, 2))
    fill(5, 0, wsec(dWf, 2, 0), wsec(dWf, 2, 1))
    fill(5, 1, wsec(dWf, 0, 0), wsec(dWf, 1, 0))
    fill(6, 0, None, wsec(dWf, 2, 2))
    fill(6, 1, wsec(dWf, 0, 1), wsec(dWf, 1, 1))
    fill(7, 1, wsec(dWf, 0, 2), wsec(dWf, 1, 2))
    wpack = wpack.reshape(128, 8 * 2 * 128).astype(f8)

    bias = np.ascontiguousarray(biases.astype(np.float32).reshape(128, 1))

    in_maps = []
    for core in range(N_CORES):
        h0 = RPC * core
        xs = np.zeros((128, NSLOT, Q, W), dtype=f8)

        def seg(src, row0, colshift=0):
            # [64, Q, W] slice of src at rows row0.., columns shifted left
            out = np.zeros((64, Q, W), dtype=src.dtype)
            if colshift == 0:
                out[:] = src[:, row0 : row0 + Q]
            else:
                out[:, :, : W - colshift] = src[:, row0 : row0 + Q, colshift:]
            return out

        xs[:64, 0] = seg(X8f, h0 + 2)            # B upper
        xs[64:, 0] = seg(X8f, h0 + 2, 1)         # B lower
        xs[:64, 1] = seg(X8f, h0)                # A upper
        xs[64:, 1] = seg(X8f, h0 + 1)            # A lower
        xs[:64, 2] = seg(dX8f, h0)               # C upper
        xs[64:, 2] = seg(dX8f, h0 + 1)           # C lower
        xs[:64, 3] = seg(dX8f, h0 + 2)           # D upper
        xs[64:, 3] = seg(dX8f, h0 + 2, 1)        # D lower
        xs[:64, 4] = seg(X8f, h0, 2)             # F upper
        xs[64:, 4] = seg(dX8f, h0, 2)            # F lower
        xs[:64, 5] = seg(X8f, h0 + 1, 2)         # G upper
        xs[64:, 5] = seg(dX8f, h0 + 1, 2)        # G lower

        in_maps.append(
            {
                "xall": xs.reshape(128, NSLOT * Q * W),
                "wpack": wpack,
                "bias": bias,
            }
        )
    return in_maps


def kernel(x, kernels, biases):
    global LAST_RESULTS
    x = np.asarray(x, dtype=np.float32)
    kernels = np.asarray(kernels, dtype=np.float32)
    biases = np.asarray(biases, dtype=np.float32)

    nc = _get_program()
    in_maps = _prep_inputs(x, kernels, biases)
    res = run_bass_kernel_spmd(nc, in_maps, core_ids=list(range(N_CORES)), trace=TRACE)
    LAST_RESULTS = res

    out = np.empty((OUT_C, N_CORES * RPC, OW), dtype=np.float32)
    for c in range(N_CORES):
        out[:, RPC * c : RPC * (c + 1), :] = (
            res.results[c]["out"].astype(np.float32).reshape(OUT_C, RPC, OW)
        )
    return np.ascontiguousarray(out[:, :OH, :])


# revision 61
# speedup vs baseline: 1.7104x; 1.0170x over previous
"""Trainium2 Bass kernel for a 3x3 VALID conv: x[64,256,256] * k[128,64,3,3] -> [128,254,254].

Strategy (fp8 DoubleRow with error compensation):
  - Shard output rows across 8 cores (32 rows each; 8*32=256 >= 254, tail padded).
  - Represent x ~= X8 + dX8 and 16w ~= W16 + dW16 (all fp8e4m3; the x16
    scale is a power of two so it is exact).  The three first-order terms
    X8*W16 + dX8*W16 + X8*dW16 reproduce the conv to ~1e-3 relative error
    (the dropped dX*dW term is ~1e-4); PSUM holds 16x the result and the
    evacuation rescales by 1/16 while adding the bias.
  - fp8 DoubleRow matmuls fuse TWO K=128 products per instruction and the
    cost model charges them at 0.5 cycles per output column, so the
    27 tap-terms (9 taps x 3 terms) fit in 8 DoubleRow instructions per
    output row = 8*0.5*254 cycles, vs 2.5*508 for the bf16 scheme.
  - Six precomputed fp8 x-layout "slots" live in one SBUF tile (slot-major)
    so a 3D AP [part, slot-pair, col] addresses each DoubleRow's moving
    data; per-slot partition halves carry the two packed taps:
      B: (X8[q+2]   | X8[q+2] shifted 1 col)   kernel-row-2 taps
      A: (X8[q]     | X8[q+1])                 kernel-rows-0/1 taps
      C: (dX8[q]    | dX8[q+1])
      D: (dX8[q+2]  | dX8[q+2] shifted 1 col)
      F: (X8[q] c+2 | dX8[q] c+2)              tap (0,2) for both variants
      G: (X8[q+1]c+2| dX8[q+1]c+2)             tap (1,2) for both variants
  - DMA queues serialize per dispatching engine, so loads fan out: slots
    B,A,C on SP, D,F,G on Pool, weights+bias on ACT; stores pair two rows
    and fan out over SP/Pool.
  - Evacuation = DVE tensor_scalar (x 1/16, + bias), bf16 out.
  - A short dummy-matmul pad keeps the PE queue busy until the first
    slices land (skips the ~1.7us DMA latency on the first real matmul).
  - Tapered tail: row 31 in two chunks, one merged store.
  - Host gathers the 8 per-core output slabs.
"""

import os
import sys

import numpy as np

for _p in ("/opt/trn_rl_repo", "/root/.axon_site/_ro/trn_rl_repo"):
    if os.path.isdir(_p) and _p not in sys.path:
        sys.path.insert(0, _p)

from concourse import bass, mybir, tile  # noqa: E402
from concourse.bass_utils import run_bass_kernel_spmd  # noqa: E402

IN_C, H, W = 64, 256, 256
KS = 3
OUT_C = 128
OH, OW = H - KS + 1, W - KS + 1  # 254, 254
N_CORES = 8
RPC = 32          # output rows computed per core
PAD_H = 259
Q = RPC
NSLOT = 6
# 7-instruction DoubleRow schedule.  kind: 'AA'/'BB' broadcast one slot to
# both DoubleRow halves (stride-0 AP) so W16 rides half 0 and dW16 half 1;
# 'CD'/'SS' use two adjacent slots.  (section, kind, column offset):
DRS = [
    (0, "AA", 0),  # X8 x (W16|dW16) taps (0,0),(1,0)
    (1, "AA", 1),  # X8 x (W16|dW16) taps (0,1),(1,1)
    (2, "AA", 2),  # X8 x (W16|dW16) taps (0,2),(1,2)
    (3, "BB", 0),  # X8 x (W16|dW16) taps (2,0),(2,1)
    (4, "CD", 0),  # dX8*W16 taps (0,0),(1,0),(2,0),(2,1)
    (5, "CD", 1),  # dX8*W16 taps (0,1),(1,1),(2,2)
    (6, "SS", 0),  # X8*W16+X8*dW16 tap (2,2); dX8*W16 taps (0,2),(1,2)
]

N_WARM = int(os.environ.get("CONV_N_WARM", "3"))
WARM_AP = int(os.environ.get("CONV_WARM_AP", "120"))
MM_DT = "fp8dr"  # informational


def _tail31():
    n2 = int(os.environ.get("CONV_TAIL2", "252"))
    if n2 >= OW:
        return [(0, OW)]
    return [(0, OW - n2), (OW - n2, n2)]


TAIL31 = _tail31()

# q-row load slice boundaries (per slot): 2-row lead, then 4-row bulk.
SLICES = [(0, 2), (2, 8), (8, 14), (14, 20), (20, 26), (26, 32)]

TRACE = False
LAST_RESULTS = None

_COMPILED = {}


def _np_dt(mdt):
    return np.dtype(mybir.dt.np(mdt))


def _np_bf16():
    return _np_dt(mybir.dt.bfloat16)


def _build_program():
    f8 = mybir.dt.float8e4
    bf = mybir.dt.bfloat16
    f32 = mybir.dt.float32
    DR = mybir.MatmulPerfMode.DoubleRow
    nc = bass.Bass()

    x_ext = nc.declare_dram_parameter("xall", [128, NSLOT * Q * W], f8, isOutput=False)
    w_ext = nc.declare_dram_parameter("wpack", [128, 7 * 2 * 128], f8, isOutput=False)
    b_ext = nc.declare_dram_parameter("bias", [128, 1], f32, isOutput=False)
    o_ext = nc.declare_dram_parameter("out", [128, RPC * OW], bf, isOutput=True)

    with tile.TileContext(nc) as tc:
        with (
            tc.tile_pool(name="wpool", bufs=1) as wpool,
            tc.tile_pool(name="xpool", bufs=1) as xpool,
            tc.tile_pool(name="pspool", bufs=4, space="PSUM") as pspool,
            tc.tile_pool(name="wmpool", bufs=1, space="PSUM") as wmpool,
            tc.tile_pool(name="opool", bufs=18) as opool,
        ):
            if N_WARM:
                wmt = wpool.tile([128, 128], bf)
                nc.vector.memset(wmt[:], 0.0)
                psw = wmpool.tile([128, WARM_AP], f32)
                for _ in range(N_WARM):
                    nc.tensor.matmul(
                        psw[:],
                        lhsT=wmt[:],
                        rhs=wmt[:, 0:WARM_AP],
                        start=True,
                        stop=True,
                    )

            wt = wpool.tile([128, 7 * 2 * 128], f8)
            xt = xpool.tile([128, NSLOT * Q * W], f8)
            bt = wpool.tile([128, 1], f32)

            # weights (2 chunks) + bias on ACT
            nc.scalar.dma_start(out=wt[:, 0 : 4 * 256], in_=w_ext[:, 0 : 4 * 256])
            nc.scalar.dma_start(out=wt[:, 4 * 256 :], in_=w_ext[:, 4 * 256 :])
            nc.scalar.dma_start(out=bt[:], in_=b_ext[:])
            wrm = wpool.tile([128, 1], mybir.dt.bfloat16)
            nc.scalar.activation(
                wrm[:], bt[:, 0:1], mybir.ActivationFunctionType.Identity
            )
            # x slots spread over three engines (fp8 slices all hit the
            # 500ns DMA floor, so two engines can't carry three slots each
            # without starving the last rows).  The late-consumed Sa/Sb
            # slots ride ACT behind the weights.
            for q0, q1 in SLICES:
                for eng, s in (
                    (nc.sync, 0),     # A
                    (nc.gpsimd, 1),   # B
                    (nc.sync, 2),     # C
                    (nc.gpsimd, 3),   # D
                    (nc.scalar, 4),   # Sa
                    (nc.scalar, 5),   # Sb
                ):
                    o = s * Q * W
                    eng.dma_start(
                        out=xt[:, o + q0 * W : o + q1 * W],
                        in_=x_ext[:, o + q0 * W : o + q1 * W],
                    )

            wv = wt[:].rearrange("p (j t m) -> p j t m", t=2, m=128)
            ov = o_ext.rearrange("p (r w) -> p r w", w=OW)
            xv = xt[:].rearrange("p (s q w) -> p s q w", s=NSLOT, w=W)

            def conv_row(ps_ap, r0, c0, ncol):
                for i, (sec, kind, off) in enumerate(DRS):
                    lo = c0 + off
                    if kind == "AA":
                        rhs = (
                            xv[:, 0, r0, lo : lo + ncol]
                            .rearrange("p (o n) -> p o n", o=1)
                            .broadcast_to([128, 2, ncol])
                        )
                    elif kind == "BB":
                        rhs = (
                            xv[:, 1, r0, lo : lo + ncol]
                            .rearrange("p (o n) -> p o n", o=1)
                            .broadcast_to([128, 2, ncol])
                        )
                    elif kind == "CD":
                        rhs = xv[:, 2:4, r0, lo : lo + ncol]
                    else:
                        rhs = xv[:, 4:6, r0, lo : lo + ncol]
                    nc.tensor.matmul(
                        ps_ap,
                        lhsT=wv[:, sec, :, :],
                        rhs=rhs,
                        start=(i == 0),
                        stop=(i == len(DRS) - 1),
                        perf_mode=DR,
                    )

            def evac(so_ap, ps_ap):
                # out = psum/16 + bias
                nc.vector.tensor_scalar(
                    so_ap,
                    ps_ap,
                    1.0 / 16.0,
                    bt[:, 0:1],
                    mybir.AluOpType.mult,
                    mybir.AluOpType.add,
                )

            def act_evac(so_ap, ps_ap):
                nc.scalar.activation(
                    so_ap,
                    ps_ap,
                    mybir.ActivationFunctionType.Identity,
                    bias=bt[:, 0:1],
                    scale=1.0 / 16.0,
                )

            # rows 0..29 as 15 store-pairs; evacs alternate DVE/ACT (the
            # 7-DR rows outpace a single evac engine), stores go to SP and
            # Pool once their slot loads drain
            for pair in range(15):
                so = opool.tile([128, 2 * OW], bf)
                for k in range(2):
                    r = 2 * pair + k
                    ps = pspool.tile([128, OW], f32)
                    conv_row(ps[:], r, 0, OW)
                    evac(so[:, k * OW : (k + 1) * OW], ps[:])
                sov = so[:].rearrange("p (b n) -> p b n", n=OW)
                st = nc.sync if pair < 8 else nc.gpsimd
                st.dma_start(
                    out=ov[:, 2 * pair : 2 * pair + 2, :], in_=sov[:, :, :]
                )

            # tail: row30 + row31 in two chunks, one merged store on SP
            sot = opool.tile([128, 2 * OW], bf)
            ps30 = pspool.tile([128, OW], f32, bufs=1)
            conv_row(ps30[:], 30, 0, OW)
            act_evac(sot[:, 0:OW], ps30[:])
            for i, (c0, ncol) in enumerate(TAIL31):
                ps31 = pspool.tile([128, ncol], f32, bufs=2)
                conv_row(ps31[:], 31, c0, ncol)
                if i == 0 and len(TAIL31) > 1:
                    evac(sot[:, OW + c0 : OW + c0 + ncol], ps31[:])
                else:
                    act_evac(sot[:, OW + c0 : OW + c0 + ncol], ps31[:])
            sotv = sot[:].rearrange("p (b n) -> p b n", n=OW)
            nc.sync.dma_start(out=ov[:, 30:32, :], in_=sotv[:, :, :])

    _split_multi_waits(nc)
    return nc


def _split_multi_waits(nc):
    """Walrus codegen accepts a single sync-wait command per instruction."""
    for fn in nc.m.functions:
        for bb in fn.blocks:
            out = []
            for inst in bb.instructions:
                si = inst.sync_info
                waits = list(si.on_wait) if si is not None and si.on_wait else []
                if len(waits) > 1:
                    for wt_ in waits[:-1]:
                        nop = mybir.InstNoOp(
                            name=nc.get_next_instruction_name(),
                            engine=inst.engine,
                        )
                        nop.sync_info = mybir.SyncInfo(on_wait=[wt_], on_update=[])
                        nc.register_instruction(nop)
                        out.append(nop)
                    inst.sync_info = mybir.SyncInfo(
                        on_wait=[waits[-1]], on_update=list(si.on_update)
                    )
                out.append(inst)
            bb.instructions = out


def _get_program(_unused=None):
    key = "v8"
    if key not in _COMPILED:
        _COMPILED[key] = _build_program()
    return _COMPILED[key]


def _prep_inputs(x, kernels, biases, _unused=None):
    f8 = _np_dt(mybir.dt.float8e4)
    bf16 = _np_dt(mybir.dt.bfloat16)

    xp = np.zeros((IN_C, PAD_H, W), dtype=np.float32)
    xp[:, :H] = x
    X8f = xp.astype(f8)
    X8 = X8f.astype(np.float32)
    dX8f = (xp - X8).astype(f8)

    w16 = kernels.astype(np.float32) * 16.0
    W16f = w16.astype(f8)
    W16 = W16f.astype(np.float32)
    dW16f = (w16 - W16).astype(f8)
    W16 = W16f.astype(np.float32)

    def wsec(wf, kh, kw):
        # [64, 128] fp8->f32 weight block transposed (chan, outch)
        return wf[:, :, kh, kw].T.astype(np.float32)

    # 7 sections x 2 halves x [128 part, 128 outch]
    wpack = np.zeros((128, 7, 2, 128), dtype=np.float32)

    def fill(sec, t, upper, lower):
        if upper is not None:
            wpack[:64, sec, t, :] = upper
        if lower is not None:
            wpack[64:, sec, t, :] = lower

    Wf, dWf = W16f, dW16f
    for kw in range(3):            # (A,A)@kw: W on half0, dW on half1
        fill(kw, 0, wsec(Wf, 0, kw), wsec(Wf, 1, kw))
        fill(kw, 1, wsec(dWf, 0, kw), wsec(dWf, 1, kw))
    fill(3, 0, wsec(Wf, 2, 0), wsec(Wf, 2, 1))     # (B,B)@0
    fill(3, 1, wsec(dWf, 2, 0), wsec(dWf, 2, 1))
    fill(4, 0, wsec(Wf, 0, 0), wsec(Wf, 1, 0))     # (C,D)@0
    fill(4, 1, wsec(Wf, 2, 0), wsec(Wf, 2, 1))
    fill(5, 0, wsec(Wf, 0, 1), wsec(Wf, 1, 1))     # (C,D)@1
    fill(5, 1, None, wsec(Wf, 2, 2))
    fill(6, 0, wsec(Wf, 2, 2), wsec(dWf, 2, 2))    # (Sa,Sb)@0
    fill(6, 1, wsec(Wf, 0, 2), wsec(Wf, 1, 2))
    wpack = wpack.reshape(128, 7 * 2 * 128).astype(f8)

    bias = np.ascontiguousarray(biases.astype(np.float32).reshape(128, 1))

    in_maps = []
    for core in range(N_CORES):
        h0 = RPC * core
        xs = np.zeros((128, NSLOT, Q, W), dtype=f8)

        def seg(src, row0, colshift=0):
            # [64, Q, W] slice of src at rows row0.., columns shifted left
            out = np.zeros((64, Q, W), dtype=src.dtype)
            if colshift == 0:
                out[:] = src[:, row0 : row0 + Q]
            else:
                out[:, :, : W - colshift] = src[:, row0 : row0 + Q, colshift:]
            return out

        xs[:64, 0] = seg(X8f, h0)                # A upper  X8[q]
        xs[64:, 0] = seg(X8f, h0 + 1)            # A lower  X8[q+1]
        xs[:64, 1] = seg(X8f, h0 + 2)            # B upper  X8[q+2]
        xs[64:, 1] = seg(X8f, h0 + 2, 1)         # B lower  X8[q+2] c+1
        xs[:64, 2] = seg(dX8f, h0)               # C upper  dX8[q]
        xs[64:, 2] = seg(dX8f, h0 + 1)           # C lower  dX8[q+1]
        xs[:64, 3] = seg(dX8f, h0 + 2)           # D upper  dX8[q+2]
        xs[64:, 3] = seg(dX8f, h0 + 2, 1)        # D lower  dX8[q+2] c+1
        xs[:64, 4] = seg(X8f, h0 + 2, 2)         # Sa upper X8[q+2] c+2
        xs[64:, 4] = seg(X8f, h0 + 2, 2)         # Sa lower (same)
        xs[:64, 5] = seg(dX8f, h0, 2)            # Sb upper dX8[q] c+2
        xs[64:, 5] = seg(dX8f, h0 + 1, 2)        # Sb lower dX8[q+1] c+2
        in_maps.append(
            {
                "xall": xs.reshape(128, NSLOT * Q * W),
                "wpack": wpack,
                "bias": bias,
            }
        )
    return in_maps


def kernel(x, kernels, biases):
    global LAST_RESULTS
    x = np.asarray(x, dtype=np.float32)
    kernels = np.asarray(kernels, dtype=np.float32)
    biases = np.asarray(biases, dtype=np.float32)

    nc = _get_program()
    in_maps = _prep_inputs(x, kernels, biases)
    res = run_bass_kernel_spmd(nc, in_maps, core_ids=list(range(N_CORES)), trace=TRACE)
    LAST_RESULTS = res

    out = np.empty((OUT_C, N_CORES * RPC, OW), dtype=np.float32)
    for c in range(N_CORES):
        out[:, RPC * c : RPC * (c + 1), :] = (
            res.results[c]["out"].astype(np.float32).reshape(OUT_C, RPC, OW)
        )
    return np.ascontiguousarray(out[:, :OH, :])


# revision 65
# speedup vs baseline: 1.7379x; 1.0161x over previous
"""Trainium2 Bass kernel for a 3x3 VALID conv: x[64,256,256] * k[128,64,3,3] -> [128,254,254].

Strategy (fp8 DoubleRow with error compensation):
  - Shard output rows across 8 cores (32 rows each; 8*32=256 >= 254, tail padded).
  - Represent x ~= X8 + dX8 and 16w ~= W16 + dW16 (all fp8e4m3; the x16
    scale is a power of two so it is exact).  The three first-order terms
    X8*W16 + dX8*W16 + X8*dW16 reproduce the conv to ~1e-3 relative error
    (the dropped dX*dW term is ~1e-4); PSUM holds 16x the result and the
    evacuation rescales by 1/16 while adding the bias.
  - fp8 DoubleRow matmuls fuse TWO K=128 products per instruction and the
    cost model charges them at 0.5 cycles per output column, so the
    27 tap-terms (9 taps x 3 terms) fit in 8 DoubleRow instructions per
    output row = 8*0.5*254 cycles, vs 2.5*508 for the bf16 scheme.
  - Six precomputed fp8 x-layout "slots" live in one SBUF tile (slot-major)
    so a 3D AP [part, slot-pair, col] addresses each DoubleRow's moving
    data; per-slot partition halves carry the two packed taps:
      B: (X8[q+2]   | X8[q+2] shifted 1 col)   kernel-row-2 taps
      A: (X8[q]     | X8[q+1])                 kernel-rows-0/1 taps
      C: (dX8[q]    | dX8[q+1])
      D: (dX8[q+2]  | dX8[q+2] shifted 1 col)
      F: (X8[q] c+2 | dX8[q] c+2)              tap (0,2) for both variants
      G: (X8[q+1]c+2| dX8[q+1]c+2)             tap (1,2) for both variants
  - DMA queues serialize per dispatching engine, so loads fan out: slots
    B,A,C on SP, D,F,G on Pool, weights+bias on ACT; stores pair two rows
    and fan out over SP/Pool.
  - Evacuation = DVE tensor_scalar (x 1/16, + bias), bf16 out.
  - A short dummy-matmul pad keeps the PE queue busy until the first
    slices land (skips the ~1.7us DMA latency on the first real matmul).
  - Tapered tail: row 31 in two chunks, one merged store.
  - Host gathers the 8 per-core output slabs.
"""

import os
import sys

import numpy as np

for _p in ("/opt/trn_rl_repo", "/root/.axon_site/_ro/trn_rl_repo"):
    if os.path.isdir(_p) and _p not in sys.path:
        sys.path.insert(0, _p)

from concourse import bass, mybir, tile  # noqa: E402
from concourse.bass_utils import run_bass_kernel_spmd  # noqa: E402

IN_C, H, W = 64, 256, 256
KS = 3
OUT_C = 128
OH, OW = H - KS + 1, W - KS + 1  # 254, 254
N_CORES = 8
RPC = 32          # output rows computed per core
PAD_H = 259
Q = RPC
NSLOT = 6
# 7-instruction DoubleRow schedule.  kind: 'AA'/'BB' broadcast one slot to
# both DoubleRow halves (stride-0 AP) so W16 rides half 0 and dW16 half 1;
# 'CD'/'SS' use two adjacent slots.  (section, kind, column offset):
DRS = [
    (0, "AA", 0),  # X8 x (W16|dW16) taps (0,0),(1,0)
    (1, "AA", 1),  # X8 x (W16|dW16) taps (0,1),(1,1)
    (2, "AA", 2),  # X8 x (W16|dW16) taps (0,2),(1,2)
    (3, "BB", 0),  # X8 x (W16|dW16) taps (2,0),(2,1)
    (4, "CD", 0),  # dX8*W16 taps (0,0),(1,0),(2,0),(2,1)
    (5, "CD", 1),  # dX8*W16 taps (0,1),(1,1),(2,2)
    (6, "SS", 0),  # X8*W16+X8*dW16 tap (2,2); dX8*W16 taps (0,2),(1,2)
]

N_WARM = int(os.environ.get("CONV_N_WARM", "3"))
WARM_AP = int(os.environ.get("CONV_WARM_AP", "120"))
MM_DT = "fp8dr"  # informational


def _tail31():
    n2 = int(os.environ.get("CONV_TAIL2", "252"))
    if n2 >= OW:
        return [(0, OW)]
    return [(0, OW - n2), (OW - n2, n2)]


TAIL31 = _tail31()

# q-row load slice boundaries (per slot): 2-row lead, then 4-row bulk.
SLICES = [(0, 5), (5, 10), (10, 15), (15, 20), (20, 25), (25, 32)]

TRACE = False
LAST_RESULTS = None

_COMPILED = {}


def _np_dt(mdt):
    return np.dtype(mybir.dt.np(mdt))


def _np_bf16():
    return _np_dt(mybir.dt.bfloat16)


def _build_program():
    f8 = mybir.dt.float8e4
    bf = mybir.dt.bfloat16
    f32 = mybir.dt.float32
    DR = mybir.MatmulPerfMode.DoubleRow
    nc = bass.Bass()

    x_ext = nc.declare_dram_parameter("xall", [128, NSLOT * Q * W], f8, isOutput=False)
    w_ext = nc.declare_dram_parameter("wpack", [128, 7 * 2 * 128], f8, isOutput=False)
    b_ext = nc.declare_dram_parameter("bias", [128, 1], f32, isOutput=False)
    o_ext = nc.declare_dram_parameter("out", [128, RPC * OW], bf, isOutput=True)

    with tile.TileContext(nc) as tc:
        with (
            tc.tile_pool(name="wpool", bufs=1) as wpool,
            tc.tile_pool(name="xpool", bufs=1) as xpool,
            tc.tile_pool(name="pspool", bufs=4, space="PSUM") as pspool,
            tc.tile_pool(name="wmpool", bufs=1, space="PSUM") as wmpool,
            tc.tile_pool(name="opool", bufs=18) as opool,
        ):
            if N_WARM:
                wmt = wpool.tile([128, 128], bf)
                nc.vector.memset(wmt[:], 0.0)
                psw = wmpool.tile([128, WARM_AP], f32)
                for _ in range(N_WARM):
                    nc.tensor.matmul(
                        psw[:],
                        lhsT=wmt[:],
                        rhs=wmt[:, 0:WARM_AP],
                        start=True,
                        stop=True,
                    )

            wt = wpool.tile([128, 7 * 2 * 128], f8)
            xt = xpool.tile([128, NSLOT * Q * W], f8)
            bt = wpool.tile([128, 1], f32)

            # weights (2 chunks) + bias on ACT
            nc.scalar.dma_start(out=wt[:, 0 : 4 * 256], in_=w_ext[:, 0 : 4 * 256])
            nc.scalar.dma_start(out=wt[:, 4 * 256 :], in_=w_ext[:, 4 * 256 :])
            nc.scalar.dma_start(out=bt[:], in_=b_ext[:])
            wrm = wpool.tile([128, 1], mybir.dt.bfloat16)
            nc.scalar.activation(
                wrm[:], bt[:, 0:1], mybir.ActivationFunctionType.Identity
            )
            # x slots spread over three engines (fp8 slices all hit the
            # 500ns DMA floor, so two engines can't carry three slots each
            # without starving the last rows).  The late-consumed Sa/Sb
            # slots ride ACT behind the weights.
            for q0, q1 in SLICES:
                for eng, s in (
                    (nc.sync, 0),     # A
                    (nc.gpsimd, 1),   # B
                    (nc.sync, 2),     # C
                    (nc.gpsimd, 3),   # D
                    (nc.scalar, 4),   # Sa
                    (nc.scalar, 5),   # Sb
                ):
                    o = s * Q * W
                    eng.dma_start(
                        out=xt[:, o + q0 * W : o + q1 * W],
                        in_=x_ext[:, o + q0 * W : o + q1 * W],
                    )

            wv = wt[:].rearrange("p (j t m) -> p j t m", t=2, m=128)
            ov = o_ext.rearrange("p (r w) -> p r w", w=OW)
            xv = xt[:].rearrange("p (s q w) -> p s q w", s=NSLOT, w=W)

            def conv_row(ps_ap, r0, c0, ncol):
                for i, (sec, kind, off) in enumerate(DRS):
                    lo = c0 + off
                    if kind == "AA":
                        rhs = (
                            xv[:, 0, r0, lo : lo + ncol]
                            .rearrange("p (o n) -> p o n", o=1)
                            .broadcast_to([128, 2, ncol])
                        )
                    elif kind == "BB":
                        rhs = (
                            xv[:, 1, r0, lo : lo + ncol]
                            .rearrange("p (o n) -> p o n", o=1)
                            .broadcast_to([128, 2, ncol])
                        )
                    elif kind == "CD":
                        rhs = xv[:, 2:4, r0, lo : lo + ncol]
                    else:
                        rhs = xv[:, 4:6, r0, lo : lo + ncol]
                    nc.tensor.matmul(
                        ps_ap,
                        lhsT=wv[:, sec, :, :],
                        rhs=rhs,
                        start=(i == 0),
                        stop=(i == len(DRS) - 1),
                        perf_mode=DR,
                    )

            def evac(so_ap, ps_ap):
                # out = psum/16 + bias
                nc.vector.tensor_scalar(
                    so_ap,
                    ps_ap,
                    1.0 / 16.0,
                    bt[:, 0:1],
                    mybir.AluOpType.mult,
                    mybir.AluOpType.add,
                )

            def act_evac(so_ap, ps_ap):
                nc.scalar.activation(
                    so_ap,
                    ps_ap,
                    mybir.ActivationFunctionType.Identity,
                    bias=bt[:, 0:1],
                    scale=1.0 / 16.0,
                )

            # rows 0..29 as 15 store-pairs; evacs alternate DVE/ACT (the
            # 7-DR rows outpace a single evac engine), stores go to SP and
            # Pool once their slot loads drain
            for pair in range(15):
                so = opool.tile([128, 2 * OW], bf)
                for k in range(2):
                    r = 2 * pair + k
                    ps = pspool.tile([128, OW], f32)
                    conv_row(ps[:], r, 0, OW)
                    evac(so[:, k * OW : (k + 1) * OW], ps[:])
                sov = so[:].rearrange("p (b n) -> p b n", n=OW)
                st = nc.sync if pair < 8 else nc.gpsimd
                st.dma_start(
                    out=ov[:, 2 * pair : 2 * pair + 2, :], in_=sov[:, :, :]
                )

            # tail: row30 + row31 in two chunks, one merged store on SP
            sot = opool.tile([128, 2 * OW], bf)
            ps30 = pspool.tile([128, OW], f32, bufs=1)
            conv_row(ps30[:], 30, 0, OW)
            act_evac(sot[:, 0:OW], ps30[:])
            for i, (c0, ncol) in enumerate(TAIL31):
                ps31 = pspool.tile([128, ncol], f32, bufs=2)
                conv_row(ps31[:], 31, c0, ncol)
                if i == 0 and len(TAIL31) > 1:
                    evac(sot[:, OW + c0 : OW + c0 + ncol], ps31[:])
                else:
                    act_evac(sot[:, OW + c0 : OW + c0 + ncol], ps31[:])
            sotv = sot[:].rearrange("p (b n) -> p b n", n=OW)
            nc.sync.dma_start(out=ov[:, 30:32, :], in_=sotv[:, :, :])

    _split_multi_waits(nc)
    return nc


def _split_multi_waits(nc):
    """Walrus codegen accepts a single sync-wait command per instruction."""
    for fn in nc.m.functions:
        for bb in fn.blocks:
            out = []
            for inst in bb.instructions:
                si = inst.sync_info
                waits = list(si.on_wait) if si is not None and si.on_wait else []
                if len(waits) > 1:
                    for wt_ in waits[:-1]:
                        nop = mybir.InstNoOp(
                            name=nc.get_next_instruction_name(),
                            engine=inst.engine,
                        )
                        nop.sync_info = mybir.SyncInfo(on_wait=[wt_], on_update=[])
                        nc.register_instruction(nop)
                        out.append(nop)
                    inst.sync_info = mybir.SyncInfo(
                        on_wait=[waits[-1]], on_update=list(si.on_update)
                    )
                out.append(inst)
            bb.instructions = out


def _get_program(_unused=None):
    key = "v8"
    if key not in _COMPILED:
        _COMPILED[key] = _build_program()
    return _COMPILED[key]


def _prep_inputs(x, kernels, biases, _unused=None):
    f8 = _np_dt(mybir.dt.float8e4)
    bf16 = _np_dt(mybir.dt.bfloat16)

    xp = np.zeros((IN_C, PAD_H, W), dtype=np.float32)
    xp[:, :H] = x
    X8f = xp.astype(f8)
    X8 = X8f.astype(np.float32)
    dX8f = (xp - X8).astype(f8)

    w16 = kernels.astype(np.float32) * 16.0
    W16f = w16.astype(f8)
    W16 = W16f.astype(np.float32)
    dW16f = (w16 - W16).astype(f8)
    W16 = W16f.astype(np.float32)

    def wsec(wf, kh, kw):
        # [64, 128] fp8->f32 weight block transposed (chan, outch)
        return wf[:, :, kh, kw].T.astype(np.float32)

    # 7 sections x 2 halves x [128 part, 128 outch]
    wpack = np.zeros((128, 7, 2, 128), dtype=np.float32)

    def fill(sec, t, upper, lower):
        if upper is not None:
            wpack[:64, sec, t, :] = upper
        if lower is not None:
            wpack[64:, sec, t, :] = lower

    Wf, dWf = W16f, dW16f
    for kw in range(3):            # (A,A)@kw: W on half0, dW on half1
        fill(kw, 0, wsec(Wf, 0, kw), wsec(Wf, 1, kw))
        fill(kw, 1, wsec(dWf, 0, kw), wsec(dWf, 1, kw))
    fill(3, 0, wsec(Wf, 2, 0), wsec(Wf, 2, 1))     # (B,B)@0
    fill(3, 1, wsec(dWf, 2, 0), wsec(dWf, 2, 1))
    fill(4, 0, wsec(Wf, 0, 0), wsec(Wf, 1, 0))     # (C,D)@0
    fill(4, 1, wsec(Wf, 2, 0), wsec(Wf, 2, 1))
    fill(5, 0, wsec(Wf, 0, 1), wsec(Wf, 1, 1))     # (C,D)@1
    fill(5, 1, None, wsec(Wf, 2, 2))
    fill(6, 0, wsec(Wf, 2, 2), wsec(dWf, 2, 2))    # (Sa,Sb)@0
    fill(6, 1, wsec(Wf, 0, 2), wsec(Wf, 1, 2))
    wpack = wpack.reshape(128, 7 * 2 * 128).astype(f8)

    bias = np.ascontiguousarray(biases.astype(np.float32).reshape(128, 1))

    in_maps = []
    for core in range(N_CORES):
        h0 = RPC * core
        xs = np.zeros((128, NSLOT, Q, W), dtype=f8)

        def seg(src, row0, colshift=0):
            # [64, Q, W] slice of src at rows row0.., columns shifted left
            out = np.zeros((64, Q, W), dtype=src.dtype)
            if colshift == 0:
                out[:] = src[:, row0 : row0 + Q]
            else:
                out[:, :, : W - colshift] = src[:, row0 : row0 + Q, colshift:]
            return out

        xs[:64, 0] = seg(X8f, h0)                # A upper  X8[q]
        xs[64:, 0] = seg(X8f, h0 + 1)            # A lower  X8[q+1]
        xs[:64, 1] = seg(X8f, h0 + 2)            # B upper  X8[q+2]
        xs[64:, 1] = seg(X8f, h0 + 2, 1)         # B lower  X8[q+2] c+1
        xs[:64, 2] = seg(dX8f, h0)               # C upper  dX8[q]
        xs[64:, 2] = seg(dX8f, h0 + 1)           # C lower  dX8[q+1]
        xs[:64, 3] = seg(dX8f, h0 + 2)           # D upper  dX8[q+2]
        xs[64:, 3] = seg(dX8f, h0 + 2, 1)        # D lower  dX8[q+2] c+1
        xs[:64, 4] = seg(X8f, h0 + 2, 2)         # Sa upper X8[q+2] c+2
        xs[64:, 4] = seg(X8f, h0 + 2, 2)         # Sa lower (same)
        xs[:64, 5] = seg(dX8f, h0, 2)            # Sb upper dX8[q] c+2
        xs[64:, 5] = seg(dX8f, h0 + 1, 2)        # Sb lower dX8[q+1] c+2
        in_maps.append(
            {
                "xall": xs.reshape(128, NSLOT * Q * W),
                "wpack": wpack,
                "bias": bias,
            }
        )
    return in_maps


def kernel(x, kernels, biases):
    global LAST_RESULTS
    x = np.asarray(x, dtype=np.float32)
    kernels = np.asarray(kernels, dtype=np.float32)
    biases = np.asarray(biases, dtype=np.float32)

    nc = _get_program()
    in_maps = _prep_inputs(x, kernels, biases)
    res = run_bass_kernel_spmd(nc, in_maps, core_ids=list(range(N_CORES)), trace=TRACE)
    LAST_RESULTS = res

    out = np.empty((OUT_C, N_CORES * RPC, OW), dtype=np.float32)
    for c in range(N_CORES):
        out[:, RPC * c : RPC * (c + 1), :] = (
            res.results[c]["out"].astype(np.float32).reshape(OUT_C, RPC, OW)
        )
    return np.ascontiguousarray(out[:, :OH, :])


# revision 73
# speedup vs baseline: 1.7393x; 1.0008x over previous
"""Trainium2 Bass kernel for a 3x3 VALID conv: x[64,256,256] * k[128,64,3,3] -> [128,254,254].

Strategy (fp8 DoubleRow with error compensation):
  - Shard output rows across 8 cores (32 rows each; 8*32=256 >= 254, tail padded).
  - Represent x ~= X8 + dX8 and 16w ~= W16 + dW16 (all fp8e4m3; the x16
    scale is a power of two so it is exact).  The three first-order terms
    X8*W16 + dX8*W16 + X8*dW16 reproduce the conv to ~1e-3 relative error
    (the dropped dX*dW term is ~1e-4); PSUM holds 16x the result and the
    evacuation rescales by 1/16 while adding the bias.
  - fp8 DoubleRow matmuls fuse TWO K=128 products per instruction and the
    cost model charges them at 0.5 cycles per output column, so the
    27 tap-terms (9 taps x 3 terms) fit in 8 DoubleRow instructions per
    output row = 8*0.5*254 cycles, vs 2.5*508 for the bf16 scheme.
  - Six precomputed fp8 x-layout "slots" live in one SBUF tile (slot-major)
    so a 3D AP [part, slot-pair, col] addresses each DoubleRow's moving
    data; per-slot partition halves carry the two packed taps:
      B: (X8[q+2]   | X8[q+2] shifted 1 col)   kernel-row-2 taps
      A: (X8[q]     | X8[q+1])                 kernel-rows-0/1 taps
      C: (dX8[q]    | dX8[q+1])
      D: (dX8[q+2]  | dX8[q+2] shifted 1 col)
      F: (X8[q] c+2 | dX8[q] c+2)              tap (0,2) for both variants
      G: (X8[q+1]c+2| dX8[q+1]c+2)             tap (1,2) for both variants
  - DMA queues serialize per dispatching engine, so loads fan out: slots
    B,A,C on SP, D,F,G on Pool, weights+bias on ACT; stores pair two rows
    and fan out over SP/Pool.
  - Evacuation = DVE tensor_scalar (x 1/16, + bias), bf16 out.
  - A short dummy-matmul pad keeps the PE queue busy until the first
    slices land (skips the ~1.7us DMA latency on the first real matmul).
  - Tapered tail: row 31 in two chunks, one merged store.
  - Host gathers the 8 per-core output slabs.
"""

import os
import sys

import numpy as np

for _p in ("/opt/trn_rl_repo", "/root/.axon_site/_ro/trn_rl_repo"):
    if os.path.isdir(_p) and _p not in sys.path:
        sys.path.insert(0, _p)

from concourse import bass, mybir, tile  # noqa: E402
from concourse.bass_utils import run_bass_kernel_spmd  # noqa: E402

IN_C, H, W = 64, 256, 256
KS = 3
OUT_C = 128
OH, OW = H - KS + 1, W - KS + 1  # 254, 254
N_CORES = 8
RPC = 32          # output rows computed per core
PAD_H = 259
Q = RPC
NSLOT = 6
# 7-instruction DoubleRow schedule.  kind: 'AA'/'BB' broadcast one slot to
# both DoubleRow halves (stride-0 AP) so W16 rides half 0 and dW16 half 1;
# 'CD'/'SS' use two adjacent slots.  (section, kind, column offset):
DRS = [
    (0, "AA", 0),  # X8 x (W16|dW16) taps (0,0),(1,0)
    (1, "AA", 1),  # X8 x (W16|dW16) taps (0,1),(1,1)
    (2, "AA", 2),  # X8 x (W16|dW16) taps (0,2),(1,2)
    (3, "BB", 0),  # X8 x (W16|dW16) taps (2,0),(2,1)
    (4, "CD", 0),  # dX8*W16 taps (0,0),(1,0),(2,0),(2,1)
    (5, "CD", 1),  # dX8*W16 taps (0,1),(1,1),(2,2)
    (6, "SS", 0),  # X8*W16+X8*dW16 tap (2,2); dX8*W16 taps (0,2),(1,2)
]

N_WARM = int(os.environ.get("CONV_N_WARM", "3"))
WARM_AP = int(os.environ.get("CONV_WARM_AP", "114"))
MM_DT = "fp8dr"  # informational


def _tail31():
    n2 = int(os.environ.get("CONV_TAIL2", "252"))
    if n2 >= OW:
        return [(0, OW)]
    return [(0, OW - n2), (OW - n2, n2)]


TAIL31 = _tail31()

# q-row load slice boundaries (per slot): 2-row lead, then 4-row bulk.
SLICES = [(0, 5), (5, 10), (10, 15), (15, 20), (20, 25), (25, 32)]

TRACE = False
LAST_RESULTS = None

_COMPILED = {}


def _np_dt(mdt):
    return np.dtype(mybir.dt.np(mdt))


def _np_bf16():
    return _np_dt(mybir.dt.bfloat16)


def _build_program():
    f8 = mybir.dt.float8e4
    bf = mybir.dt.bfloat16
    f32 = mybir.dt.float32
    DR = mybir.MatmulPerfMode.DoubleRow
    nc = bass.Bass()

    x_ext = nc.declare_dram_parameter("xall", [128, NSLOT * Q * W], f8, isOutput=False)
    w_ext = nc.declare_dram_parameter("wpack", [128, 7 * 2 * 128], f8, isOutput=False)
    b_ext = nc.declare_dram_parameter("bias", [128, 1], f32, isOutput=False)
    o_ext = nc.declare_dram_parameter("out", [128, RPC * OW], bf, isOutput=True)

    with tile.TileContext(nc) as tc:
        with (
            tc.tile_pool(name="wpool", bufs=1) as wpool,
            tc.tile_pool(name="xpool", bufs=1) as xpool,
            tc.tile_pool(name="pspool", bufs=4, space="PSUM") as pspool,
            tc.tile_pool(name="wmpool", bufs=1, space="PSUM") as wmpool,
            tc.tile_pool(name="opool", bufs=18) as opool,
        ):
            if N_WARM:
                wmt = wpool.tile([128, 128], bf)
                nc.vector.memset(wmt[:], 0.0)
                psw = wmpool.tile([128, WARM_AP], f32)
                for _ in range(N_WARM):
                    nc.tensor.matmul(
                        psw[:],
                        lhsT=wmt[:],
                        rhs=wmt[:, 0:WARM_AP],
                        start=True,
                        stop=True,
                    )

            wt = wpool.tile([128, 7 * 2 * 128], f8)
            xt = xpool.tile([128, NSLOT * Q * W], f8)
            bt = wpool.tile([128, 1], f32)

            # weights (2 chunks) + bias on ACT
            nc.scalar.dma_start(out=wt[:, 0 : 4 * 256], in_=w_ext[:, 0 : 4 * 256])
            nc.scalar.dma_start(out=wt[:, 4 * 256 :], in_=w_ext[:, 4 * 256 :])
            nc.scalar.dma_start(out=bt[:], in_=b_ext[:])
            wrm = wpool.tile([128, 1], mybir.dt.bfloat16)
            nc.scalar.activation(
                wrm[:], bt[:, 0:1], mybir.ActivationFunctionType.Identity
            )
            # x slots spread over three engines (fp8 slices all hit the
            # 500ns DMA floor, so two engines can't carry three slots each
            # without starving the last rows).  The late-consumed Sa/Sb
            # slots ride ACT behind the weights.
            for q0, q1 in SLICES:
                for eng, s in (
                    (nc.sync, 0),     # A
                    (nc.gpsimd, 1),   # B
                    (nc.sync, 2),     # C
                    (nc.gpsimd, 3),   # D
                    (nc.scalar, 4),   # Sa
                    (nc.scalar, 5),   # Sb
                ):
                    o = s * Q * W
                    eng.dma_start(
                        out=xt[:, o + q0 * W : o + q1 * W],
                        in_=x_ext[:, o + q0 * W : o + q1 * W],
                    )

            wv = wt[:].rearrange("p (j t m) -> p j t m", t=2, m=128)
            ov = o_ext.rearrange("p (r w) -> p r w", w=OW)
            xv = xt[:].rearrange("p (s q w) -> p s q w", s=NSLOT, w=W)

            def conv_row(ps_ap, r0, c0, ncol):
                for i, (sec, kind, off) in enumerate(DRS):
                    lo = c0 + off
                    if kind == "AA":
                        rhs = (
                            xv[:, 0, r0, lo : lo + ncol]
                            .rearrange("p (o n) -> p o n", o=1)
                            .broadcast_to([128, 2, ncol])
                        )
                    elif kind == "BB":
                        rhs = (
                            xv[:, 1, r0, lo : lo + ncol]
                            .rearrange("p (o n) -> p o n", o=1)
                            .broadcast_to([128, 2, ncol])
                        )
                    elif kind == "CD":
                        rhs = xv[:, 2:4, r0, lo : lo + ncol]
                    else:
                        rhs = xv[:, 4:6, r0, lo : lo + ncol]
                    nc.tensor.matmul(
                        ps_ap,
                        lhsT=wv[:, sec, :, :],
                        rhs=rhs,
                        start=(i == 0),
                        stop=(i == len(DRS) - 1),
                        perf_mode=DR,
                    )

            def evac(so_ap, ps_ap):
                # out = psum/16 + bias
                nc.vector.tensor_scalar(
                    so_ap,
                    ps_ap,
                    1.0 / 16.0,
                    bt[:, 0:1],
                    mybir.AluOpType.mult,
                    mybir.AluOpType.add,
                )

            def act_evac(so_ap, ps_ap):
                nc.scalar.activation(
                    so_ap,
                    ps_ap,
                    mybir.ActivationFunctionType.Identity,
                    bias=bt[:, 0:1],
                    scale=1.0 / 16.0,
                )

            # rows 0..29 as 15 store-pairs; evacs alternate DVE/ACT (the
            # 7-DR rows outpace a single evac engine), stores go to SP and
            # Pool once their slot loads drain
            for pair in range(15):
                so = opool.tile([128, 2 * OW], bf)
                for k in range(2):
                    r = 2 * pair + k
                    ps = pspool.tile([128, OW], f32)
                    conv_row(ps[:], r, 0, OW)
                    evac(so[:, k * OW : (k + 1) * OW], ps[:])
                sov = so[:].rearrange("p (b n) -> p b n", n=OW)
                st = nc.sync if pair < 8 else nc.gpsimd
                st.dma_start(
                    out=ov[:, 2 * pair : 2 * pair + 2, :], in_=sov[:, :, :]
                )

            # tail: row30 + row31 in two chunks, one merged store on SP
            sot = opool.tile([128, 2 * OW], bf)
            ps30 = pspool.tile([128, OW], f32, bufs=1)
            conv_row(ps30[:], 30, 0, OW)
            act_evac(sot[:, 0:OW], ps30[:])
            for i, (c0, ncol) in enumerate(TAIL31):
                ps31 = pspool.tile([128, ncol], f32, bufs=2)
                conv_row(ps31[:], 31, c0, ncol)
                if i == 0 and len(TAIL31) > 1:
                    evac(sot[:, OW + c0 : OW + c0 + ncol], ps31[:])
                else:
                    act_evac(sot[:, OW + c0 : OW + c0 + ncol], ps31[:])
            sotv = sot[:].rearrange("p (b n) -> p b n", n=OW)
            nc.sync.dma_start(out=ov[:, 30:32, :], in_=sotv[:, :, :])

    _split_multi_waits(nc)
    return nc


def _split_multi_waits(nc):
    """Walrus codegen accepts a single sync-wait command per instruction."""
    for fn in nc.m.functions:
        for bb in fn.blocks:
            out = []
            for inst in bb.instructions:
                si = inst.sync_info
                waits = list(si.on_wait) if si is not None and si.on_wait else []
                if len(waits) > 1:
                    for wt_ in waits[:-1]:
                        nop = mybir.InstNoOp(
                            name=nc.get_next_instruction_name(),
                            engine=inst.engine,
                        )
                        nop.sync_info = mybir.SyncInfo(on_wait=[wt_], on_update=[])
                        nc.register_instruction(nop)
                        out.append(nop)
                    inst.sync_info = mybir.SyncInfo(
                        on_wait=[waits[-1]], on_update=list(si.on_update)
                    )
                out.append(inst)
            bb.instructions = out


def _get_program(_unused=None):
    key = "v8"
    if key not in _COMPILED:
        _COMPILED[key] = _build_program()
    return _COMPILED[key]


def _prep_inputs(x, kernels, biases, _unused=None):
    f8 = _np_dt(mybir.dt.float8e4)
    bf16 = _np_dt(mybir.dt.bfloat16)

    xp = np.zeros((IN_C, PAD_H, W), dtype=np.float32)
    xp[:, :H] = x
    X8f = xp.astype(f8)
    X8 = X8f.astype(np.float32)
    dX8f = (xp - X8).astype(f8)

    w16 = kernels.astype(np.float32) * 16.0
    W16f = w16.astype(f8)
    W16 = W16f.astype(np.float32)
    dW16f = (w16 - W16).astype(f8)
    W16 = W16f.astype(np.float32)

    def wsec(wf, kh, kw):
        # [64, 128] fp8->f32 weight block transposed (chan, outch)
        return wf[:, :, kh, kw].T.astype(np.float32)

    # 7 sections x 2 halves x [128 part, 128 outch]
    wpack = np.zeros((128, 7, 2, 128), dtype=np.float32)

    def fill(sec, t, upper, lower):
        if upper is not None:
            wpack[:64, sec, t, :] = upper
        if lower is not None:
            wpack[64:, sec, t, :] = lower

    Wf, dWf = W16f, dW16f
    for kw in range(3):            # (A,A)@kw: W on half0, dW on half1
        fill(kw, 0, wsec(Wf, 0, kw), wsec(Wf, 1, kw))
        fill(kw, 1, wsec(dWf, 0, kw), wsec(dWf, 1, kw))
    fill(3, 0, wsec(Wf, 2, 0), wsec(Wf, 2, 1))     # (B,B)@0
    fill(3, 1, wsec(dWf, 2, 0), wsec(dWf, 2, 1))
    fill(4, 0, wsec(Wf, 0, 0), wsec(Wf, 1, 0))     # (C,D)@0
    fill(4, 1, wsec(Wf, 2, 0), wsec(Wf, 2, 1))
    fill(5, 0, wsec(Wf, 0, 1), wsec(Wf, 1, 1))     # (C,D)@1
    fill(5, 1, None, wsec(Wf, 2, 2))
    fill(6, 0, wsec(Wf, 2, 2), wsec(dWf, 2, 2))    # (Sa,Sb)@0
    fill(6, 1, wsec(Wf, 0, 2), wsec(Wf, 1, 2))
    wpack = wpack.reshape(128, 7 * 2 * 128).astype(f8)

    bias = np.ascontiguousarray(biases.astype(np.float32).reshape(128, 1))

    in_maps = []
    for core in range(N_CORES):
        h0 = RPC * core
        xs = np.zeros((128, NSLOT, Q, W), dtype=f8)

        def seg(src, row0, colshift=0):
            # [64, Q, W] slice of src at rows row0.., columns shifted left
            out = np.zeros((64, Q, W), dtype=src.dtype)
            if colshift == 0:
                out[:] = src[:, row0 : row0 + Q]
            else:
                out[:, :, : W - colshift] = src[:, row0 : row0 + Q, colshift:]
            return out

        xs[:64, 0] = seg(X8f, h0)                # A upper  X8[q]
        xs[64:, 0] = seg(X8f, h0 + 1)            # A lower  X8[q+1]
        xs[:64, 1] = seg(X8f, h0 + 2)            # B upper  X8[q+2]
        xs[64:, 1] = seg(X8f, h0 + 2, 1)         # B lower  X8[q+2] c+1
        xs[:64, 2] = seg(dX8f, h0)               # C upper  dX8[q]
        xs[64:, 2] = seg(dX8f, h0 + 1)           # C lower  dX8[q+1]
        xs[:64, 3] = seg(dX8f, h0 + 2)           # D upper  dX8[q+2]
        xs[64:, 3] = seg(dX8f, h0 + 2, 1)        # D lower  dX8[q+2] c+1
        xs[:64, 4] = seg(X8f, h0 + 2, 2)         # Sa upper X8[q+2] c+2
        xs[64:, 4] = seg(X8f, h0 + 2, 2)         # Sa lower (same)
        xs[:64, 5] = seg(dX8f, h0, 2)            # Sb upper dX8[q] c+2
        xs[64:, 5] = seg(dX8f, h0 + 1, 2)        # Sb lower dX8[q+1] c+2
        in_maps.append(
            {
                "xall": xs.reshape(128, NSLOT * Q * W),
                "wpack": wpack,
                "bias": bias,
            }
        )
    return in_maps


def kernel(x, kernels, biases):
    global LAST_RESULTS
    x = np.asarray(x, dtype=np.float32)
    kernels = np.asarray(kernels, dtype=np.float32)
    biases = np.asarray(biases, dtype=np.float32)

    nc = _get_program()
    in_maps = _prep_inputs(x, kernels, biases)
    res = run_bass_kernel_spmd(nc, in_maps, core_ids=list(range(N_CORES)), trace=TRACE)
    LAST_RESULTS = res

    out = np.empty((OUT_C, N_CORES * RPC, OW), dtype=np.float32)
    for c in range(N_CORES):
        out[:, RPC * c : RPC * (c + 1), :] = (
            res.results[c]["out"].astype(np.float32).reshape(OUT_C, RPC, OW)
        )
    return np.ascontiguousarray(out[:, :OH, :])
